# revision 1
# baseline (speedup 1.0000x reference)
"""GNN message passing (nn_OPID_78769700208710) on 8 TRN2 NeuronCores.

Strategy: the 6-relation edge lists are combined on host into one sparse
operator M (w[e] = sign_r * softplus(g_r) * val[e]), materialized as a dense
fp16 matrix A [N_pad, N_pad] (N_pad = 20480).  Propagation h_{k+1} =
a_k*h0 + (1-a_k)*(h @ A) runs 6 steps on device.

Sharding: destination-column model parallelism.  Core c owns dst columns
[c*2560, (c+1)*2560) and streams its A slice (panels of [128 src, 2560 dst]
fp16) from HBM each step; matmuls accumulate msg in PSUM.  Steps 1-5 produce
msg in node-partition layout ([128 dst, 64 batch] per dst-block) so the next
step's stationary operand (h windows, [128 src, 64 b]) needs no transpose;
h slices are exchanged between steps with an in-kernel AllGather.  Step 6
produces msg in batch-partition layout [64, 2560] feeding the decode, which
runs fully on-device: a K=4 matmul folds W1+b1 (4th input row is ones), ACT
relu, then a column-stationary matmul against W2 sums over H.  cell_emb is
added AFTER the relu in the reference, so it passes linearly through W2 and
is folded host-side into a per-output-column bias together with b2.
"""

import numpy as np

N = 20000
NP = 20480          # padded nodes: 160 windows * 128
W = 160             # src windows of 128
B = 64              # batch
CORES = 8
NLOC = NP // CORES  # 2560 dst nodes per core
WLOC = NLOC // 128  # 20 dst blocks per core
H = 64
STEPS = 6
SIGNS = (1.0, -1.0, 1.0, -1.0, 1.0, -1.0)

_CACHE = {}


def _np_softplus(x):
    return np.log1p(np.exp(-np.abs(x))) + np.maximum(x, 0.0)


def _np_sigmoid(x):
    return 1.0 / (1.0 + np.exp(-x))


def _build_program(NP=NP, debug=False, compile_=True):
    """Build + compile the (input-independent) Bass program once."""
    key = ("nc", NP, debug)
    if key in _CACHE:
        return _CACHE[key]
    W = NP // 128
    NLOC = NP // CORES
    WLOC = NLOC // 128

    import concourse.bacc as bacc
    import concourse.mybir as mybir
    from concourse import tile

    f16 = mybir.dt.float16
    f32 = mybir.dt.float32
    AF = mybir.ActivationFunctionType
    OP = mybir.AluOpType

    nc = bacc.Bacc(
        "TRN2",
        target_bir_lowering=False,
        debug=False,
        enable_asserts=False,
        num_devices=CORES,
    )

    a2 = nc.dram_tensor("a2", [W, 128, NLOC], f16, kind="ExternalInput")
    h0t16 = nc.dram_tensor("h0t16", [128, W * B], f16, kind="ExternalInput")
    h0t = nc.dram_tensor("h0t", [128, WLOC * B], f32, kind="ExternalInput")
    x4b = nc.dram_tensor("x4b", [B, 4 * NLOC], f16, kind="ExternalInput")
    w1bT = nc.dram_tensor("w1bT", [4, B * H], f16, kind="ExternalInput")
    w2sc = nc.dram_tensor("w2sc", [H, 1], f32, kind="ExternalInput")
    b2bc = nc.dram_tensor("b2bc", [128, WLOC * B], f32, kind="ExternalInput")
    alph = nc.dram_tensor("alph", [128, 2 * STEPS], f32, kind="ExternalInput")
    y = nc.dram_tensor("y", [B, NLOC], f32, kind="ExternalOutput")
    if debug:
        dbg_h = nc.dram_tensor("dbg_h", [STEPS - 1, 128, W * B], f16, kind="ExternalOutput")
        dbg_x = nc.dram_tensor("dbg_x", [B, 4 * NLOC], f16, kind="ExternalOutput")

    NCHUNK = NLOC // 512  # 5

    with tile.TileContext(nc) as tc:
        with (
            tc.tile_pool(name="const", bufs=1) as constp,
            tc.tile_pool(name="apan", bufs=4) as apanp,
            tc.tile_pool(name="hslice", bufs=2) as hslicep,
            tc.tile_pool(name="tmp", bufs=4) as tmpp,
            tc.tile_pool(name="dec", bufs=2) as decp,
            tc.tile_pool(name="hds", bufs=4) as hdsp,
            tc.tile_pool(name="ysb", bufs=2) as ysbp,
            tc.tile_pool(name="dram", bufs=1, space="DRAM") as dramp,
        ):
            # --- persistent SBUF state ---
            h_sb = constp.tile([128, W * B], f16, tag="h_sb")
            h0t_sb = constp.tile([128, WLOC * B], f32, tag="h0t")
            alph_sb = constp.tile([128, 2 * STEPS], f32, tag="alph")
            w2_sb = constp.tile([H, 1], f32, tag="w2")
            b2_sb = constp.tile([128, WLOC * B], f32, tag="b2")
            w1b_sb = constp.tile([4, B * H], f16, tag="w1b")
            w2c_sb = constp.tile([H, 1], f16, tag="w2c")
            xsb = constp.tile([B, 4 * NLOC], f16, tag="xsb")

            nc.sync.dma_start(h_sb[:], h0t16.ap())
            nc.sync.dma_start(h0t_sb[:], h0t.ap())
            nc.sync.dma_start(alph_sb[:], alph.ap())
            nc.sync.dma_start(w2_sb[:], w2sc.ap())
            nc.sync.dma_start(b2_sb[:], b2bc.ap())
            nc.sync.dma_start(w1b_sb[:], w1bT.ap())
            nc.sync.dma_start(xsb[:], x4b.ap())
            nc.vector.tensor_copy(w2c_sb[:], w2_sb[:])

            # DRAM bounce buffers for the per-step h exchange
            bi = dramp.tile([128, WLOC * B], f16, tag="bi")
            bo = dramp.tile([CORES, 128, WLOC * B], f16, tag="bo")
            xd = dramp.tile([B, 4 * NLOC], f16, tag="xd")

            # ---------------- propagation steps 1..5 ----------------
            prop = tc.tile_pool(name="psprop", bufs=1, space="PSUM")
            ps15p = ps6p = prop.__enter__()
            for k in range(STEPS - 1):
                ps = [ps15p.tile([128, 512], f32, tag=f"ps15_{i}", name=f"ps15_{i}") for i in range(3)]
                h16s = hslicep.tile([128, WLOC * B], f16, tag="h16s")

                for w in range(W):
                    ap = apanp.tile([128, NLOC], f16, tag="apan")
                    nc.sync.dma_start(ap[:], a2.ap()[w])
                    for d in range(WLOC):
                        # one accumulation group per 2KB PSUM bank: start only
                        # on the bank's first matmul, stop on its last; other
                        # column-ranges are initialized via pending-zero bytes
                        nc.tensor.matmul(
                            ps[d // 8][:, (d % 8) * B : (d % 8 + 1) * B],
                            lhsT=ap[:, d * 128 : (d + 1) * 128],
                            rhs=h_sb[:, w * B : (w + 1) * B],
                            start=(w == 0 and d % 8 == 0),
                            stop=(w == W - 1 and (d % 8 == 7 or d == WLOC - 1)),
                        )

                # epilogue: h_new = a*h0 + (1-a)*msg, emitted as fp16
                for d in range(WLOC):
                    h0a = tmpp.tile([128, B], f32, tag="h0a")
                    nc.scalar.activation(
                        h0a[:],
                        h0t_sb[:, d * B : (d + 1) * B],
                        AF.Copy,
                        scale=alph_sb[:, k : k + 1],
                    )
                    nc.vector.scalar_tensor_tensor(
                        h16s[:, d * B : (d + 1) * B],
                        ps[d // 8][:, (d % 8) * B : (d % 8 + 1) * B],
                        alph_sb[:, STEPS + k : STEPS + k + 1],
                        h0a[:],
                        OP.mult,
                        OP.add,
                    )

                # exchange: slice -> DRAM -> AllGather -> full h_sb
                nc.sync.dma_start(bi[:], h16s[:])
                nc.gpsimd.collective_compute(
                    "AllGather",
                    OP.bypass,
                    replica_groups=[list(range(CORES))],
                    ins=[bi.opt()],
                    outs=[bo.opt()],
                )
                nc.sync.dma_start(
                    h_sb[:].rearrange("p (c f) -> p c f", c=CORES),
                    bo[:].rearrange("c p f -> p c f"),
                )
                if debug:
                    nc.sync.dma_start(dbg_h.ap()[k], h_sb[:])

            # ---------------- step 6: batch-partition output ----------------
            ps6 = [ps6p.tile([B, 512], f32, tag=f"ps6_{j}", name=f"ps6_{j}") for j in range(NCHUNK)]
            for w in range(W):
                ap = apanp.tile([128, NLOC], f16, tag="apan")
                nc.sync.dma_start(ap[:], a2.ap()[w])
                for j in range(NCHUNK):
                    nc.tensor.matmul(
                        ps6[j][:, :],
                        lhsT=h_sb[:, w * B : (w + 1) * B],
                        rhs=ap[:, j * 512 : (j + 1) * 512],
                        start=(w == 0),
                        stop=(w == W - 1),
                    )

            # epilogue 6 in batch layout, written into xsb row 2 (h6, fp16)
            k5 = STEPS - 1
            for j in range(NCHUNK):
                h0a6 = tmpp.tile([B, 512], f32, tag="h0a6")
                nc.scalar.activation(
                    h0a6[:],
                    xsb[:, NLOC + j * 512 : NLOC + (j + 1) * 512],
                    AF.Copy,
                    scale=alph_sb[:B, k5 : k5 + 1],
                )
                nc.vector.scalar_tensor_tensor(
                    xsb[:, 2 * NLOC + j * 512 : 2 * NLOC + (j + 1) * 512],
                    ps6[j][:, :],
                    alph_sb[:B, STEPS + k5 : STEPS + k5 + 1],
                    h0a6[:],
                    OP.mult,
                    OP.add,
                )

            prop.__exit__(None, None, None)

            # ---------------- decode ----------------
            decps = tc.tile_pool(name="psdec", bufs=1, space="PSUM")
            psAp = ps2p = decps.__enter__()
            nc.sync.dma_start(xd[:], xsb[:])
            if debug:
                nc.sync.dma_start(dbg_x.ap(), xsb[:])

            NQ = 8          # batch rounds
            BQ = B // NQ    # 8 batch rows per round
            ps2_tiles = [ps2p.tile([128, 512], f32, tag=f"ps2_{i}", name=f"ps2_{i}") for i in range(3)]
            ncols_done = 0
            ysb_flushed = 0
            NCOLS_TOT = B * NLOC // 128  # 1280

            for q in range(NQ):
                xT4 = decp.tile([4, BQ * NLOC], f16, tag="xT4")
                # gather [f, b, n] for this batch block from DRAM
                nc.sync.dma_start(
                    xT4[:].rearrange("f (b n) -> f b n", b=BQ),
                    xd[:].rearrange("b (f n) -> f b n", f=4)[:, q * BQ : (q + 1) * BQ, :],
                )
                for bl in range(BQ):
                    b = q * BQ + bl
                    for c5 in range(NCHUNK):
                        psA = psAp.tile([H, 512], f32, tag="psA", bufs=4)
                        nc.tensor.matmul(
                            psA[:],
                            lhsT=w1b_sb[:, b * H : (b + 1) * H],
                            rhs=xT4[
                                :, bl * NLOC + c5 * 512 : bl * NLOC + (c5 + 1) * 512
                            ],
                            start=True,
                            stop=True,
                        )
                        hds = hdsp.tile([H, 512], f16, tag="hds")
                        nc.scalar.activation(hds[:], psA[:], AF.Relu)
                        for i in range(4):
                            col = ncols_done % 512
                            ti = ncols_done // 512
                            nc.tensor.matmul(
                                ps2_tiles[ti][:, col : col + 1],
                                lhsT=hds[:, i * 128 : (i + 1) * 128],
                                rhs=w2c_sb[:],
                                start=True,
                                stop=True,
                            )
                            ncols_done += 1
                            if ncols_done % 512 == 0 or ncols_done == NCOLS_TOT:
                                nt = ncols_done - ysb_flushed
                                ysb = ysbp.tile([128, 512], f32, tag="ysb")
                                nc.vector.scalar_tensor_tensor(
                                    ysb[:, :nt],
                                    ps2_tiles[ti][:, :nt],
                                    1.0,
                                    b2_sb[:, ysb_flushed:ncols_done],
                                    OP.mult,
                                    OP.add,
                                )
                                dst = (
                                    y.ap()
                                    .rearrange("b n -> (b n)")[
                                        ysb_flushed * 128 : ncols_done * 128
                                    ]
                                    .rearrange("(f p) -> p f", p=128)
                                )
                                nc.sync.dma_start(dst, ysb[:, :nt])
                                ysb_flushed = ncols_done
            decps.__exit__(None, None, None)

    if compile_:
        nc.compile()
    _CACHE[key] = nc
    return nc


def kernel(
    ctl_base,
    u_raw,
    g_logits,
    alpha_logits,
    cell_emb,
    W1,
    b1,
    W2,
    b2,
    edge_val,
    edge_src,
    edge_dst,
    cell_idx,
):
    from concourse.bass_utils import run_bass_kernel_spmd

    ctl_base = np.asarray(ctl_base)
    u_raw = np.asarray(u_raw)
    cell_emb = np.asarray(cell_emb)
    W1 = np.asarray(W1)
    b1 = np.asarray(b1)
    W2 = np.asarray(W2)
    b2 = np.asarray(b2)
    edge_val = np.asarray(edge_val)
    edge_src = np.asarray(edge_src)
    edge_dst = np.asarray(edge_dst)
    cell_idx = np.asarray(cell_idx)

    g = _np_softplus(np.asarray(g_logits, np.float64))
    alphas = _np_sigmoid(np.asarray(alpha_logits, np.float64))

    # dense combined operator A[src, dst]
    A = np.zeros((NP, NP), np.float32)
    for r in range(6):
        w = (SIGNS[r] * g[r]) * np.asarray(edge_val[r], np.float64)
        np.add.at(A, (edge_src[r], edge_dst[r]), w.astype(np.float32))

    u_pad = np.zeros((B, NP), np.float32)
    u_pad[:, :N] = u_raw
    ctl_pad = np.zeros((B, NP), np.float32)
    ctl_pad[:, :N] = ctl_base

    # full transposed h0 in window layout: [p, w*B + b] = u[b, w*128+p]
    h0t16_full = np.ascontiguousarray(
        u_pad.reshape(B, W, 128).transpose(2, 1, 0).reshape(128, W * B)
    ).astype(np.float16)

    alph_np = np.zeros((128, 2 * STEPS), np.float32)
    alph_np[:, :STEPS] = alphas.astype(np.float32)
    alph_np[:, STEPS:] = (1.0 - alphas).astype(np.float32)

    cemb_rows = cell_emb[cell_idx]  # [B, H]
    w1bT_np = np.zeros((4, B * H), np.float16)
    for f in range(3):
        w1bT_np[f] = np.tile(W1[f].astype(np.float16), B)
    w1bT_np[3] = np.tile(b1.astype(np.float16), B)

    w2sc_np = np.ascontiguousarray(W2.reshape(H, 1)).astype(np.float32)
    # reference adds cell_emb AFTER the relu; it passes linearly through W2:
    # y += cemb[b] @ W2.  Fold per-batch constant + b2 into a per-column bias
    # (ps2 column col -> batch b = col // WLOC).
    ccb = (cemb_rows.astype(np.float64) @ W2.astype(np.float64).reshape(H)).astype(np.float32)  # [B]
    ncols_tot = B * WLOC
    bias_cols = (np.repeat(ccb, WLOC) + np.float32(b2.reshape(-1)[0])).astype(np.float32)  # [1280]
    b2bc_np = np.broadcast_to(bias_cols[None, :], (128, ncols_tot)).copy()

    nc = _build_program()

    W_, NLOC_, WLOC_ = W, NLOC, WLOC
    in_maps = []
    for c in range(CORES):
        sl = slice(c * NLOC, (c + 1) * NLOC)
        a2_c = np.ascontiguousarray(A.reshape(W, 128, NP)[:, :, sl]).astype(np.float16)
        h0t_c = np.ascontiguousarray(
            u_pad[:, sl].reshape(B, WLOC, 128).transpose(2, 1, 0).reshape(128, WLOC * B)
        ).astype(np.float32)
        x4b_c = np.zeros((B, 4, NLOC), np.float16)
        x4b_c[:, 0, :] = ctl_pad[:, sl].astype(np.float16)
        x4b_c[:, 1, :] = u_pad[:, sl].astype(np.float16)
        x4b_c[:, 3, :] = np.float16(1.0)
        in_maps.append(
            {
                "a2": a2_c,
                "h0t16": h0t16_full,
                "h0t": h0t_c,
                "x4b": x4b_c.reshape(B, 4 * NLOC),
                "w1bT": w1bT_np,
                "w2sc": w2sc_np,
                "b2bc": b2bc_np,
                "alph": alph_np,
            }
        )

    _CACHE["in_maps"] = in_maps
    res = run_bass_kernel_spmd(nc, in_maps, core_ids=list(range(CORES)))
    out = np.concatenate([res.results[c]["y"] for c in range(CORES)], axis=1)
    return np.ascontiguousarray(out[:, :N]).astype(np.float32)



# revision 29
# speedup vs baseline: 11093.2251x; 11093.2251x over previous
"""GNN message passing (nn_OPID_78769700208710) on 8 TRN2 NeuronCores.

The 6-relation edge lists are combined on host into one sparse operator
(w[e] = sign_r * softplus(g_r) * val[e]) and materialized dense in fp16.
Propagation h_{k+1} = a_k*h0 + (1-a_k)*(h @ A) runs 6 steps on device;
decode (per-node MLP) is fused into step 6.

Sharding: destination-column model parallelism — core c owns dst columns
[c*2560, (c+1)*2560).  A's slice is stored dst-major as 20 "column panels"
[128 src x (160 win * 128 dst)] so each dst block's PSUM accumulator
completes early in the step, letting the inter-step h exchange and the
decode overlap compute.

Panel materialization is split between two engines running concurrently:
  - DMA streams most quarter-panels from HBM, and
  - GPSIMD local_scatter densifies the rest from an SBUF-resident COO copy
    of the same fp16 values (A is ~0.6% dense so the COO fits in SBUF),
    adding a second materialization pipe on an otherwise idle engine.

h exchange: each step's new h slice is AllGathered in 4 groups of 5 dst
blocks.  Window order is permuted (sigma) to group-arrival order, and deep
quarter/PSUM buffering absorbs the exchange latency at step starts.

Decode: the step-6 alpha-mix is folded into W1 on host (h0 = u_raw), so
msg6 feeds the MLP directly.  X uses a partition-spread layout
([128, 1280] per step: partition f*32+r holds X_f for columns r*16+t of
each 512-column chunk) with a zero-padded ct=128 stationary W1, so X fills
are cheap 128-partition DMAs.  cell_emb passes linearly through W2 and is
folded (with b2) into a per-partition bias.
"""

import numpy as np

N = 20000
NP = 20480          # padded nodes: 160 windows * 128
W = 160             # src windows of 128
B = 64              # batch
CORES = 8
NLOC = NP // CORES  # 2560 dst nodes per core
WLOC = NLOC // 128  # 20 dst blocks per core
H = 64
STEPS = 6
SIGNS = (1.0, -1.0, 1.0, -1.0, 1.0, -1.0)

NG = 4              # exchange groups per step
BPG = WLOC // NG    # 5 dst blocks per group
# position i in sigma-order <-> global window SIGMA[i]
SIGMA = [20 * p + BPG * g + j for g in range(NG) for p in range(CORES) for j in range(BPG)]

NQ = 4              # quarters per column panel
QW = W // NQ        # 40 window-positions per quarter
CHUNK = 1280        # local_scatter chunk (columns) — must be <= 2046
NCH_Q = QW * 128 // CHUNK  # 4 chunks per quarter
P_PAD = 32          # padded nonzeros per (chunk, partition-row)

# quarters (k = d*NQ + q) with COO staged in SBUF for gpsimd local_scatter.
# Steps 1-5: only blocks d%5 in {2,3,4} — the Pool engine is blocked ~31us by
# the AllGather right when blocks d%5 in {0,1} of the next group materialize,
# so those are always DMA-sourced.  Step 6 has no collectives: spread set.
_KS48 = [d * NQ + q for d in range(WLOC) if d % BPG >= 2 for q in range(NQ)]
SCAT_P = [k for i, k in enumerate(_KS48) if (i * 23) % len(_KS48) < 23]
SCAT_6 = [k for k in range(WLOC * NQ) if (k * 32) % (WLOC * NQ) < 32]
SCAT_KS = sorted(set(SCAT_P) | set(SCAT_6))
SCAT_RANK = {kq: i for i, kq in enumerate(SCAT_KS)}
SET_P = frozenset(SCAT_P)
SET_6 = frozenset(SCAT_6)
NCK = len(SCAT_KS) * NCH_Q  # COO chunks per core

XCOLS = BPG * 128 * B       # 40960 X columns per group
YC_G = XCOLS // 128         # 320 y (ps2) columns per group

_CACHE = {}

import os as _os
_NO_SCAT = _os.environ.get("NO_SCAT", "0") == "1"


def _np_softplus(x):
    return np.log1p(np.exp(-np.abs(x))) + np.maximum(x, 0.0)


def _np_sigmoid(x):
    return 1.0 / (1.0 + np.exp(-x))


def _build_program(compile_=True, debug=False):
    key = ("nc2", debug)
    if key in _CACHE:
        return _CACHE[key]

    import concourse.bacc as bacc
    import concourse.mybir as mybir
    from concourse import tile

    f16 = mybir.dt.float16
    f32 = mybir.dt.float32
    i16 = mybir.dt.int16
    AF = mybir.ActivationFunctionType
    OP = mybir.AluOpType

    nc = bacc.Bacc(
        "TRN2",
        target_bir_lowering=False,
        debug=False,
        enable_asserts=False,
        num_devices=CORES,
    )

    a2T = nc.dram_tensor("a2T", [WLOC, 128, W * 128], f16, kind="ExternalInput")
    h0sig = nc.dram_tensor("h0sig", [128, W * B], f16, kind="ExternalInput")
    h0t = nc.dram_tensor("h0t", [128, WLOC * B], f32, kind="ExternalInput")
    coov = nc.dram_tensor("coov", [128, NCK * P_PAD], f16, kind="ExternalInput")
    cooi = nc.dram_tensor("cooi", [128, NCK * P_PAD], i16, kind="ExternalInput")
    x4bP = nc.dram_tensor("x4bP", [128, NG * XCOLS // 32], f16, kind="ExternalInput")
    w1x = nc.dram_tensor("w1x", [128, 32 * H], f16, kind="ExternalInput")
    w2c = nc.dram_tensor("w2c", [H, 1], f16, kind="ExternalInput")
    b2v = nc.dram_tensor("b2v", [128, NG * YC_G], f32, kind="ExternalInput")
    alph = nc.dram_tensor("alph", [128, 2 * STEPS], f32, kind="ExternalInput")
    y = nc.dram_tensor("y", [B, NLOC], f32, kind="ExternalOutput")
    if debug:
        dbg_h = nc.dram_tensor("dbg_h", [STEPS - 1, 128, W * B], f16, kind="ExternalOutput")
        dbg_x = nc.dram_tensor("dbg_x", [128, NG * XCOLS // 32], f16, kind="ExternalOutput")

    XW = XCOLS // 32            # 1280 xP columns per group

    with tile.TileContext(nc) as tc:
        with (
            tc.tile_pool(name="const", bufs=1) as constp,
            tc.tile_pool(name="qp", bufs=10) as qpool,
            tc.tile_pool(name="tmp", bufs=2) as tmpp,
            tc.tile_pool(name="big", bufs=2) as bip,
            tc.tile_pool(name="hds", bufs=3) as hdsp,
            tc.tile_pool(name="ysb", bufs=2) as ysbp,
            tc.tile_pool(name="accps", bufs=5, space="PSUM") as accp,
            tc.tile_pool(name="psa", bufs=2, space="PSUM") as psap,
            tc.tile_pool(name="ps2", bufs=1, space="PSUM") as ps2p,
            tc.tile_pool(name="dram", bufs=2, space="DRAM") as dramp,
        ):
            # ---- persistent SBUF state ----
            # double-buffered by step parity: step k reads set k%2, the
            # in-step exchange writes set (k+1)%2 for the next step
            h_slab = [
                [
                    constp.tile([128, QW * B], f16, tag=f"hsl{s}{g}", name=f"hsl{s}{g}")
                    for g in range(NG)
                ]
                for s in range(2)
            ]
            h0t_sb = constp.tile([128, WLOC * B], f32, tag="h0t")
            coov_sb = constp.tile([128, NCK * P_PAD], f16, tag="coov")
            cooi_sb = constp.tile([128, NCK * P_PAD], i16, tag="cooi")
            alph_sb = constp.tile([128, 2 * STEPS], f32, tag="alph")
            w1x_sb = constp.tile([128, 32 * H], f16, tag="w1x")
            w2c_sb = constp.tile([H, 1], f16, tag="w2c")
            b2v_sb = constp.tile([128, NG * YC_G], f32, tag="b2v")
            xP = constp.tile([128, NG * XW], f16, tag="xP")

            for g in range(NG):
                nc.sync.dma_start(
                    h_slab[0][g][:], h0sig.ap()[:, g * QW * B : (g + 1) * QW * B]
                )
            nc.sync.dma_start(h0t_sb[:], h0t.ap())
            nc.sync.dma_start(coov_sb[:], coov.ap())
            nc.sync.dma_start(cooi_sb[:], cooi.ap())
            nc.sync.dma_start(alph_sb[:], alph.ap())
            nc.sync.dma_start(w1x_sb[:], w1x.ap())
            nc.sync.dma_start(w2c_sb[:], w2c.ap())
            nc.sync.dma_start(b2v_sb[:], b2v.ap())
            nc.sync.dma_start(xP[:], x4bP.ap())

            # DRAM bounce for the step-6 msg -> X row reshuffle
            xrow2 = dramp.tile([WLOC, 128, B], f16, tag="xrow2", bufs=1)

            for k in range(STEPS):
                for d in range(WLOC):
                    g = d // BPG
                    acc = accp.tile([128, B], f32, tag="acc")
                    for q in range(NQ):
                        kq = d * NQ + q
                        qt = qpool.tile([128, QW * 128], f16, tag="qp")
                        use_scat = (not _NO_SCAT) and kq in (
                            SET_6 if k == STEPS - 1 else SET_P
                        )
                        if use_scat:
                            ck0 = SCAT_RANK[kq] * NCH_Q
                            for t in range(NCH_Q):
                                ck = ck0 + t
                                nc.gpsimd.local_scatter(
                                    qt[:, t * CHUNK : (t + 1) * CHUNK],
                                    coov_sb[:, ck * P_PAD : (ck + 1) * P_PAD],
                                    cooi_sb[:, ck * P_PAD : (ck + 1) * P_PAD],
                                    128,
                                    CHUNK,
                                    P_PAD,
                                )
                        else:
                            nc.sync.dma_start(
                                qt[:], a2T.ap()[d][:, q * QW * 128 : (q + 1) * QW * 128]
                            )
                        for i in range(QW):
                            nc.tensor.matmul(
                                acc[:],
                                lhsT=qt[:, i * 128 : (i + 1) * 128],
                                rhs=h_slab[k % 2][q][:, i * B : (i + 1) * B],
                                start=(q == 0 and i == 0),
                                stop=(q == NQ - 1 and i == QW - 1),
                            )

                    if k < STEPS - 1:
                        # epilogue: h_new = a*h0 + (1-a)*msg, fp16
                        if d % BPG == 0:
                            bi_sb = bip.tile([128, BPG * B], f16, tag="bi")
                        h0a = tmpp.tile([128, B], f32, tag="h0a")
                        nc.scalar.activation(
                            h0a[:],
                            h0t_sb[:, d * B : (d + 1) * B],
                            AF.Copy,
                            scale=alph_sb[:, k : k + 1],
                        )
                        nc.vector.scalar_tensor_tensor(
                            bi_sb[:, (d % BPG) * B : (d % BPG + 1) * B],
                            acc[:],
                            alph_sb[:, STEPS + k : STEPS + k + 1],
                            h0a[:],
                            OP.mult,
                            OP.add,
                        )
                        if d % BPG == BPG - 1:
                            bi_d = dramp.tile([128, BPG * B], f16, tag="bi_d")
                            bo_d = dramp.tile([CORES, 128, BPG * B], f16, tag="bo_d")
                            nc.sync.dma_start(bi_d[:], bi_sb[:])
                            nc.gpsimd.collective_compute(
                                "AllGather",
                                OP.bypass,
                                replica_groups=[list(range(CORES))],
                                ins=[bi_d.opt()],
                                outs=[bo_d.opt()],
                            )
                            nc.sync.dma_start(
                                h_slab[(k + 1) % 2][g][:].rearrange(
                                    "p (c f) -> p c f", c=CORES
                                ),
                                bo_d[:].rearrange("c p f -> p c f"),
                            )
                            if debug:
                                nc.sync.dma_start(
                                    dbg_h.ap()[k][:, g * QW * B : (g + 1) * QW * B],
                                    h_slab[(k + 1) % 2][g][:],
                                )
                    else:
                        # step 6: stage raw msg6 (alpha folded into W1) to DRAM
                        st16 = tmpp.tile([128, B], f16, tag="st16")
                        nc.scalar.activation(st16[:], acc[:], AF.Copy)
                        nc.sync.dma_start(xrow2[:][d], st16[:])

                        if d % BPG == BPG - 1:
                            # ---- decode for group g ----
                            # scatter msg6 into xP partitions 64..95 (f=2 rows)
                            nc.sync.dma_start(
                                xP[64:96, g * XW : (g + 1) * XW].rearrange(
                                    "r (c t) -> r c t", t=16
                                ),
                                xrow2[:]
                                .rearrange("d p f -> (d p f)")[
                                    g * XCOLS : (g + 1) * XCOLS
                                ]
                                .rearrange("(c r t) -> r c t", r=32, t=16),
                            )
                            ps2 = ps2p.tile([128, YC_G], f32, tag="ps2")
                            for c8 in range(XCOLS // 512):
                                psA = psap.tile([H, 512], f32, tag="psA")
                                rhs = xP[:, g * XW + c8 * 16 : g * XW + (c8 + 1) * 16]
                                for r in range(32):
                                    nc.tensor.matmul(
                                        psA[:, r * 16 : (r + 1) * 16],
                                        lhsT=w1x_sb[:, r * H : (r + 1) * H],
                                        rhs=rhs,
                                        start=True,
                                        stop=True,
                                    )
                                hds = hdsp.tile([H, 512], f16, tag="hds")
                                nc.scalar.activation(hds[:], psA[:], AF.Relu)
                                for i in range(4):
                                    col = c8 * 4 + i
                                    nc.tensor.matmul(
                                        ps2[:, col : col + 1],
                                        lhsT=hds[:, i * 128 : (i + 1) * 128],
                                        rhs=w2c_sb[:],
                                        start=True,
                                        stop=True,
                                    )
                            ysb = ysbp.tile([128, YC_G], f32, tag="ysb")
                            nc.vector.scalar_tensor_tensor(
                                ysb[:],
                                ps2[:],
                                1.0,
                                b2v_sb[:, g * YC_G : (g + 1) * YC_G],
                                OP.mult,
                                OP.add,
                            )
                            if debug:
                                nc.sync.dma_start(
                                    dbg_x.ap()[:, g * XW : (g + 1) * XW],
                                    xP[:, g * XW : (g + 1) * XW],
                                )
                            ysp = y.ap().rearrange("b (f t) -> f t b", t=2)
                            for t in range(2):
                                nc.sync.dma_start(
                                    ysp[g * YC_G : (g + 1) * YC_G][:, t, :].rearrange(
                                        "f b -> b f"
                                    ),
                                    ysb[t * 64 : (t + 1) * 64, :],
                                )

    if compile_:
        nc.compile()
    _CACHE[key] = nc
    return nc


def _host_prep(ctl_base, u_raw, g_logits, alpha_logits, cell_emb,
               W1, b1, W2, b2, edge_val, edge_src, edge_dst, cell_idx):
    g = _np_softplus(np.asarray(g_logits, np.float64))
    alphas = _np_sigmoid(np.asarray(alpha_logits, np.float64))

    A = np.zeros((NP, NP), np.float32)
    for r in range(6):
        w = (SIGNS[r] * g[r]) * np.asarray(edge_val[r], np.float64)
        np.add.at(A, (edge_src[r], edge_dst[r]), w.astype(np.float32))

    u_pad = np.zeros((B, NP), np.float32)
    u_pad[:, :N] = u_raw
    ctl_pad = np.zeros((B, NP), np.float32)
    ctl_pad[:, :N] = ctl_base

    sig = np.asarray(SIGMA)
    # h0 in sigma window layout: [p, i*B+b] = u[b, SIGMA[i]*128+p]
    h0sig_np = np.ascontiguousarray(
        u_pad.reshape(B, W, 128)[:, sig, :].transpose(2, 1, 0).reshape(128, W * B)
    ).astype(np.float16)

    alph_np = np.zeros((128, 2 * STEPS), np.float32)
    alph_np[:, :STEPS] = alphas.astype(np.float32)
    alph_np[:, STEPS:] = (1.0 - alphas).astype(np.float32)

    a5 = alphas[STEPS - 1]
    w1r = np.zeros((4, H), np.float16)
    w1r[0] = W1[0].astype(np.float16)
    w1r[1] = (W1[1].astype(np.float64) + a5 * W1[2].astype(np.float64)).astype(np.float16)
    w1r[2] = ((1.0 - a5) * W1[2].astype(np.float64)).astype(np.float16)
    w1r[3] = b1.astype(np.float16)
    # zero-padded ct=128 stationary operand: w1x[f*32+rho, r*64+h] nonzero
    # only when rho == r
    w1x_np = np.zeros((128, 32 * H), np.float16)
    for r in range(32):
        for f in range(4):
            w1x_np[f * 32 + r, r * H : (r + 1) * H] = w1r[f]

    w2c_np = np.ascontiguousarray(W2.reshape(H, 1)).astype(np.float16)

    # cell_emb passes linearly through W2: per-batch constant + b2, and with
    # n-major X columns the ps2 partition p corresponds to batch b = p % 64.
    ccb = (cell_emb[cell_idx].astype(np.float64) @ W2.astype(np.float64).reshape(H)).astype(np.float32)
    bias_p = (ccb[np.arange(128) % B] + np.float32(np.asarray(b2).reshape(-1)[0]))
    b2v_np = np.broadcast_to(bias_p[:, None], (128, NG * YC_G)).copy().astype(np.float32)

    in_maps = []
    for c in range(CORES):
        sl = slice(c * NLOC, (c + 1) * NLOC)
        Acore = A[:, sl].reshape(W, 128, WLOC, 128)[sig]      # [i, p, d, j]
        a2T_c = np.ascontiguousarray(Acore.transpose(2, 1, 0, 3)).reshape(
            WLOC, 128, W * 128
        ).astype(np.float16)

        # COO for the scatter-capable quarters, from the SAME fp16 values
        coov_c = np.zeros((NCK, 128, P_PAD), np.float16)
        cooi_c = -np.ones((NCK, 128, P_PAD), np.int16)
        for si, kq in enumerate(SCAT_KS):
            d, q = divmod(kq, NQ)
            sub = a2T_c[d][:, q * QW * 128 : (q + 1) * QW * 128].reshape(128, NCH_Q, CHUNK)
            pp, tt, cc = np.nonzero(sub)
            rows = pp * NCH_Q + tt
            cnt = np.bincount(rows, minlength=128 * NCH_Q)
            assert cnt.max() <= P_PAD, f"chunk row overflow: {cnt.max()} > {P_PAD}"
            offs = np.zeros(128 * NCH_Q, np.int64)
            np.cumsum(cnt[:-1], out=offs[1:])
            pos = np.arange(len(rows)) - offs[rows]
            for t in range(NCH_Q):
                m = tt == t
                ck = si * NCH_Q + t
                cooi_c[ck, pp[m], pos[m]] = cc[m].astype(np.int16)
                coov_c[ck, pp[m], pos[m]] = sub[pp[m], t, cc[m]]
        coov_dev = np.ascontiguousarray(coov_c.transpose(1, 0, 2)).reshape(128, NCK * P_PAD)
        cooi_dev = np.ascontiguousarray(cooi_c.transpose(1, 0, 2)).reshape(128, NCK * P_PAD)

        h0t_c = np.ascontiguousarray(
            u_pad[:, sl].reshape(B, WLOC, 128).transpose(2, 1, 0).reshape(128, WLOC * B)
        ).astype(np.float32)

        # X rows (ctl, u, -, ones) in the partition-spread layout:
        # x4bP[f*32+r, g*1280 + c*16 + t] = X_f[group g, col c*512 + r*16 + t]
        x4bP_c = np.zeros((128, NG * XW_HOST), np.float16)
        for f, row in ((0, ctl_pad[:, sl]), (1, u_pad[:, sl]), (3, None)):
            if row is None:
                flat = np.ones(NLOC * B, np.float32)
            else:
                flat = np.ascontiguousarray(row.T).reshape(-1)  # [n*B + b]
            v = flat.reshape(NG, 80, 32, 16).transpose(2, 0, 1, 3).reshape(32, NG * XW_HOST)
            x4bP_c[f * 32 : (f + 1) * 32] = v.astype(np.float16)

        in_maps.append(
            {
                "a2T": a2T_c,
                "h0sig": h0sig_np,
                "h0t": h0t_c,
                "coov": coov_dev,
                "cooi": cooi_dev,
                "x4bP": x4bP_c,
                "w1x": w1x_np,
                "w2c": w2c_np,
                "b2v": b2v_np,
                "alph": alph_np,
            }
        )
    return in_maps


XW_HOST = XCOLS // 32


def kernel(
    ctl_base,
    u_raw,
    g_logits,
    alpha_logits,
    cell_emb,
    W1,
    b1,
    W2,
    b2,
    edge_val,
    edge_src,
    edge_dst,
    cell_idx,
):
    from concourse.bass_utils import run_bass_kernel_spmd

    args = dict(
        ctl_base=np.asarray(ctl_base), u_raw=np.asarray(u_raw),
        g_logits=np.asarray(g_logits), alpha_logits=np.asarray(alpha_logits),
        cell_emb=np.asarray(cell_emb), W1=np.asarray(W1), b1=np.asarray(b1),
        W2=np.asarray(W2), b2=np.asarray(b2), edge_val=np.asarray(edge_val),
        edge_src=np.asarray(edge_src), edge_dst=np.asarray(edge_dst),
        cell_idx=np.asarray(cell_idx),
    )
    in_maps = _host_prep(**args)
    nc = _build_program()
    _CACHE["in_maps"] = in_maps
    res = run_bass_kernel_spmd(nc, in_maps, core_ids=list(range(CORES)))
    out = np.concatenate([res.results[c]["y"] for c in range(CORES)], axis=1)
    return np.ascontiguousarray(out[:, :N]).astype(np.float32)


# revision 36
# speedup vs baseline: 11317.5587x; 1.0202x over previous
"""GNN message passing (nn_OPID_78769700208710) on 8 TRN2 NeuronCores.

The 6-relation edge lists are combined on host into one sparse operator
(w[e] = sign_r * softplus(g_r) * val[e]) and materialized dense in fp16.
Propagation h_{k+1} = a_k*h0 + (1-a_k)*(h @ A) runs 6 steps on device;
decode (per-node MLP) is fused into step 6.

Sharding: destination-column model parallelism — core c owns dst columns
[c*2560, (c+1)*2560).  A's slice is stored dst-major as 20 "column panels"
[128 src x (160 win * 128 dst)] so each dst block's PSUM accumulator
completes early in the step, letting the inter-step h exchange and the
decode overlap compute.

Panel materialization is split between two engines running concurrently:
  - DMA streams most quarter-panels from HBM, and
  - GPSIMD local_scatter densifies the rest from an SBUF-resident COO copy
    of the same fp16 values (A is ~0.6% dense so the COO fits in SBUF),
    adding a second materialization pipe on an otherwise idle engine.

h exchange: each step's new h slice is AllGathered in 4 groups of 5 dst
blocks.  Window order is permuted (sigma) to group-arrival order, and deep
quarter/PSUM buffering absorbs the exchange latency at step starts.

Decode: the step-6 alpha-mix is folded into W1 on host (h0 = u_raw), so
msg6 feeds the MLP directly.  X uses a partition-spread layout
([128, 1280] per step: partition f*32+r holds X_f for columns r*16+t of
each 512-column chunk) with a zero-padded ct=128 stationary W1, so X fills
are cheap 128-partition DMAs.  cell_emb passes linearly through W2 and is
folded (with b2) into a per-partition bias.
"""

import numpy as np

N = 20000
NP = 20480          # padded nodes: 160 windows * 128
W = 160             # src windows of 128
B = 64              # batch
CORES = 8
NLOC = NP // CORES  # 2560 dst nodes per core
WLOC = NLOC // 128  # 20 dst blocks per core
H = 64
STEPS = 6
SIGNS = (1.0, -1.0, 1.0, -1.0, 1.0, -1.0)

NG = 4              # decode groups per step (xP/y granularity)
BPG = WLOC // NG    # 5 dst blocks per decode group
NG_EX = 2           # exchange groups per step (collective granularity)
BPG_EX = WLOC // NG_EX  # 10 dst blocks per exchange group
# position i in sigma-order <-> global window SIGMA[i]
SIGMA = [
    20 * p + BPG_EX * g + j
    for g in range(NG_EX)
    for p in range(CORES)
    for j in range(BPG_EX)
]

NQ = 4              # quarters per column panel
QW = W // NQ        # 40 window-positions per quarter
CHUNK = 1280        # local_scatter chunk (columns) — must be <= 2046
NCH_Q = QW * 128 // CHUNK  # 4 chunks per quarter
P_PAD = 32          # padded nonzeros per (chunk, partition-row)

# quarters (k = d*NQ + q) with COO staged in SBUF for gpsimd local_scatter.
# Steps 1-5: only blocks d%10 in {3..9} — the Pool engine is blocked ~48us by
# the AllGather right when blocks d%10 in {0,1,2} of the next group
# materialize, so those are always DMA-sourced.  Step 6 has no collectives:
# spread set, sized so DMA and gpsimd finish together.
_KS_OK = [d * NQ + q for d in range(WLOC) if d % BPG_EX >= 3 for q in range(NQ)]
SCAT_P = [k for i, k in enumerate(_KS_OK) if (i * 27) % len(_KS_OK) < 27]
SCAT_6 = [k for k in range(WLOC * NQ) if (k * 38) % (WLOC * NQ) < 38]
SCAT_KS = sorted(set(SCAT_P) | set(SCAT_6))
SCAT_RANK = {kq: i for i, kq in enumerate(SCAT_KS)}
SET_P = frozenset(SCAT_P)
SET_6 = frozenset(SCAT_6)
NCK = len(SCAT_KS) * NCH_Q  # COO chunks per core

XCOLS = BPG * 128 * B       # 40960 X columns per group
YC_G = XCOLS // 128         # 320 y (ps2) columns per group

_CACHE = {}

import os as _os
_NO_SCAT = _os.environ.get("NO_SCAT", "0") == "1"


def _np_softplus(x):
    return np.log1p(np.exp(-np.abs(x))) + np.maximum(x, 0.0)


def _np_sigmoid(x):
    return 1.0 / (1.0 + np.exp(-x))


def _build_program(compile_=True, debug=False):
    key = ("nc2", debug)
    if key in _CACHE:
        return _CACHE[key]

    import concourse.bacc as bacc
    import concourse.mybir as mybir
    from concourse import tile

    f16 = mybir.dt.float16
    f32 = mybir.dt.float32
    i16 = mybir.dt.int16
    AF = mybir.ActivationFunctionType
    OP = mybir.AluOpType

    nc = bacc.Bacc(
        "TRN2",
        target_bir_lowering=False,
        debug=False,
        enable_asserts=False,
        num_devices=CORES,
    )

    a2T = nc.dram_tensor("a2T", [WLOC, 128, W * 128], f16, kind="ExternalInput")
    h0sig = nc.dram_tensor("h0sig", [128, W * B], f16, kind="ExternalInput")
    h0t = nc.dram_tensor("h0t", [128, WLOC * B], f32, kind="ExternalInput")
    coov = nc.dram_tensor("coov", [128, NCK * P_PAD], f16, kind="ExternalInput")
    cooi = nc.dram_tensor("cooi", [128, NCK * P_PAD], i16, kind="ExternalInput")
    x4bP = nc.dram_tensor("x4bP", [128, NG * XCOLS // 32], f16, kind="ExternalInput")
    w1x = nc.dram_tensor("w1x", [128, 32 * H], f16, kind="ExternalInput")
    w2c = nc.dram_tensor("w2c", [H, 1], f16, kind="ExternalInput")
    b2v = nc.dram_tensor("b2v", [128, NG * YC_G], f32, kind="ExternalInput")
    alph = nc.dram_tensor("alph", [128, 2 * STEPS], f32, kind="ExternalInput")
    y = nc.dram_tensor("y", [B, NLOC], f32, kind="ExternalOutput")
    if debug:
        dbg_h = nc.dram_tensor("dbg_h", [STEPS - 1, 128, W * B], f16, kind="ExternalOutput")
        dbg_x = nc.dram_tensor("dbg_x", [128, NG * XCOLS // 32], f16, kind="ExternalOutput")

    XW = XCOLS // 32            # 1280 xP columns per group

    with tile.TileContext(nc) as tc:
        with (
            tc.tile_pool(name="const", bufs=1) as constp,
            tc.tile_pool(name="qp", bufs=10) as qpool,
            tc.tile_pool(name="tmp", bufs=2) as tmpp,
            tc.tile_pool(name="big", bufs=2) as bip,
            tc.tile_pool(name="hds", bufs=3) as hdsp,
            tc.tile_pool(name="ysb", bufs=2) as ysbp,
            tc.tile_pool(name="accps", bufs=5, space="PSUM") as accp,
            tc.tile_pool(name="psa", bufs=2, space="PSUM") as psap,
            tc.tile_pool(name="ps2", bufs=1, space="PSUM") as ps2p,
            tc.tile_pool(name="dram", bufs=2, space="DRAM") as dramp,
        ):
            # ---- persistent SBUF state ----
            # double-buffered by step parity: step k reads set k%2, the
            # in-step exchange writes set (k+1)%2 for the next step
            h_slab = [
                [
                    constp.tile([128, QW * B], f16, tag=f"hsl{s}{g}", name=f"hsl{s}{g}")
                    for g in range(NG)
                ]
                for s in range(2)
            ]
            h0t_sb = constp.tile([128, WLOC * B], f32, tag="h0t")
            coov_sb = constp.tile([128, NCK * P_PAD], f16, tag="coov")
            cooi_sb = constp.tile([128, NCK * P_PAD], i16, tag="cooi")
            alph_sb = constp.tile([128, 2 * STEPS], f32, tag="alph")
            w1x_sb = constp.tile([128, 32 * H], f16, tag="w1x")
            w2c_sb = constp.tile([H, 1], f16, tag="w2c")
            b2v_sb = constp.tile([128, NG * YC_G], f32, tag="b2v")
            xP = constp.tile([128, NG * XW], f16, tag="xP")

            for g in range(NG):
                nc.sync.dma_start(
                    h_slab[0][g][:], h0sig.ap()[:, g * QW * B : (g + 1) * QW * B]
                )
            nc.sync.dma_start(h0t_sb[:], h0t.ap())
            nc.sync.dma_start(coov_sb[:], coov.ap())
            nc.sync.dma_start(cooi_sb[:], cooi.ap())
            nc.sync.dma_start(alph_sb[:], alph.ap())
            nc.sync.dma_start(w1x_sb[:], w1x.ap())
            nc.sync.dma_start(w2c_sb[:], w2c.ap())
            nc.sync.dma_start(b2v_sb[:], b2v.ap())
            nc.sync.dma_start(xP[:], x4bP.ap())

            # DRAM bounce for the step-6 msg -> X row reshuffle
            xrow2 = dramp.tile([WLOC, 128, B], f16, tag="xrow2", bufs=1)

            for k in range(STEPS):
                for d in range(WLOC):
                    g = d // BPG
                    acc = accp.tile([128, B], f32, tag="acc")
                    for q in range(NQ):
                        kq = d * NQ + q
                        qt = qpool.tile([128, QW * 128], f16, tag="qp")
                        use_scat = (not _NO_SCAT) and kq in (
                            SET_6 if k == STEPS - 1 else SET_P
                        )
                        if use_scat:
                            ck0 = SCAT_RANK[kq] * NCH_Q
                            for t in range(NCH_Q):
                                ck = ck0 + t
                                nc.gpsimd.local_scatter(
                                    qt[:, t * CHUNK : (t + 1) * CHUNK],
                                    coov_sb[:, ck * P_PAD : (ck + 1) * P_PAD],
                                    cooi_sb[:, ck * P_PAD : (ck + 1) * P_PAD],
                                    128,
                                    CHUNK,
                                    P_PAD,
                                )
                        else:
                            nc.sync.dma_start(
                                qt[:], a2T.ap()[d][:, q * QW * 128 : (q + 1) * QW * 128]
                            )
                        for i in range(QW):
                            nc.tensor.matmul(
                                acc[:],
                                lhsT=qt[:, i * 128 : (i + 1) * 128],
                                rhs=h_slab[k % 2][q][:, i * B : (i + 1) * B],
                                start=(q == 0 and i == 0),
                                stop=(q == NQ - 1 and i == QW - 1),
                            )

                    if k < STEPS - 1:
                        # epilogue: h_new = a*h0 + (1-a)*msg, fp16
                        if d % BPG_EX == 0:
                            bi_sb = bip.tile([128, BPG_EX * B], f16, tag="bi")
                        h0a = tmpp.tile([128, B], f32, tag="h0a")
                        nc.scalar.activation(
                            h0a[:],
                            h0t_sb[:, d * B : (d + 1) * B],
                            AF.Copy,
                            scale=alph_sb[:, k : k + 1],
                        )
                        nc.vector.scalar_tensor_tensor(
                            bi_sb[:, (d % BPG_EX) * B : (d % BPG_EX + 1) * B],
                            acc[:],
                            alph_sb[:, STEPS + k : STEPS + k + 1],
                            h0a[:],
                            OP.mult,
                            OP.add,
                        )
                        if d % BPG_EX == BPG_EX - 1:
                            ge = d // BPG_EX
                            bi_d = dramp.tile([128, BPG_EX * B], f16, tag="bi_d")
                            bo_d = dramp.tile(
                                [CORES, 128, BPG_EX * B], f16, tag="bo_d"
                            )
                            nc.sync.dma_start(bi_d[:], bi_sb[:])
                            nc.gpsimd.collective_compute(
                                "AllGather",
                                OP.bypass,
                                replica_groups=[list(range(CORES))],
                                ins=[bi_d.opt()],
                                outs=[bo_d.opt()],
                            )
                            # group ge covers positions [80*ge, 80*ge+80) =
                            # slabs 2*ge (cores 0-3) and 2*ge+1 (cores 4-7)
                            for half in range(2):
                                slab = h_slab[(k + 1) % 2][2 * ge + half]
                                nc.sync.dma_start(
                                    slab[:].rearrange("p (c f) -> p c f", c=4),
                                    bo_d[:][4 * half : 4 * half + 4].rearrange(
                                        "c p f -> p c f"
                                    ),
                                )
                                if debug:
                                    nc.sync.dma_start(
                                        dbg_h.ap()[k][
                                            :,
                                            (2 * ge + half)
                                            * QW
                                            * B : (2 * ge + half + 1)
                                            * QW
                                            * B,
                                        ],
                                        slab[:],
                                    )
                    else:
                        # step 6: stage raw msg6 (alpha folded into W1) to DRAM
                        st16 = tmpp.tile([128, B], f16, tag="st16")
                        nc.scalar.activation(st16[:], acc[:], AF.Copy)
                        nc.sync.dma_start(xrow2[:][d], st16[:])

                        if d % BPG == BPG - 1:
                            # ---- decode for group g ----
                            # scatter msg6 into xP partitions 64..95 (f=2 rows)
                            nc.sync.dma_start(
                                xP[64:96, g * XW : (g + 1) * XW].rearrange(
                                    "r (c t) -> r c t", t=16
                                ),
                                xrow2[:]
                                .rearrange("d p f -> (d p f)")[
                                    g * XCOLS : (g + 1) * XCOLS
                                ]
                                .rearrange("(c r t) -> r c t", r=32, t=16),
                            )
                            ps2 = ps2p.tile([128, YC_G], f32, tag="ps2")
                            for c8 in range(XCOLS // 512):
                                psA = psap.tile([H, 512], f32, tag="psA")
                                rhs = xP[:, g * XW + c8 * 16 : g * XW + (c8 + 1) * 16]
                                for r in range(32):
                                    nc.tensor.matmul(
                                        psA[:, r * 16 : (r + 1) * 16],
                                        lhsT=w1x_sb[:, r * H : (r + 1) * H],
                                        rhs=rhs,
                                        start=True,
                                        stop=True,
                                    )
                                hds = hdsp.tile([H, 512], f16, tag="hds")
                                nc.scalar.activation(hds[:], psA[:], AF.Relu)
                                for i in range(4):
                                    col = c8 * 4 + i
                                    nc.tensor.matmul(
                                        ps2[:, col : col + 1],
                                        lhsT=hds[:, i * 128 : (i + 1) * 128],
                                        rhs=w2c_sb[:],
                                        start=True,
                                        stop=True,
                                    )
                            ysb = ysbp.tile([128, YC_G], f32, tag="ysb")
                            nc.vector.scalar_tensor_tensor(
                                ysb[:],
                                ps2[:],
                                1.0,
                                b2v_sb[:, g * YC_G : (g + 1) * YC_G],
                                OP.mult,
                                OP.add,
                            )
                            if debug:
                                nc.sync.dma_start(
                                    dbg_x.ap()[:, g * XW : (g + 1) * XW],
                                    xP[:, g * XW : (g + 1) * XW],
                                )
                            ysp = y.ap().rearrange("b (f t) -> f t b", t=2)
                            for t in range(2):
                                nc.sync.dma_start(
                                    ysp[g * YC_G : (g + 1) * YC_G][:, t, :].rearrange(
                                        "f b -> b f"
                                    ),
                                    ysb[t * 64 : (t + 1) * 64, :],
                                )

    if compile_:
        nc.compile()
    _CACHE[key] = nc
    return nc


def _host_prep(ctl_base, u_raw, g_logits, alpha_logits, cell_emb,
               W1, b1, W2, b2, edge_val, edge_src, edge_dst, cell_idx):
    g = _np_softplus(np.asarray(g_logits, np.float64))
    alphas = _np_sigmoid(np.asarray(alpha_logits, np.float64))

    A = np.zeros((NP, NP), np.float32)
    for r in range(6):
        w = (SIGNS[r] * g[r]) * np.asarray(edge_val[r], np.float64)
        np.add.at(A, (edge_src[r], edge_dst[r]), w.astype(np.float32))

    u_pad = np.zeros((B, NP), np.float32)
    u_pad[:, :N] = u_raw
    ctl_pad = np.zeros((B, NP), np.float32)
    ctl_pad[:, :N] = ctl_base

    sig = np.asarray(SIGMA)
    # h0 in sigma window layout: [p, i*B+b] = u[b, SIGMA[i]*128+p]
    h0sig_np = np.ascontiguousarray(
        u_pad.reshape(B, W, 128)[:, sig, :].transpose(2, 1, 0).reshape(128, W * B)
    ).astype(np.float16)

    alph_np = np.zeros((128, 2 * STEPS), np.float32)
    alph_np[:, :STEPS] = alphas.astype(np.float32)
    alph_np[:, STEPS:] = (1.0 - alphas).astype(np.float32)

    a5 = alphas[STEPS - 1]
    w1r = np.zeros((4, H), np.float16)
    w1r[0] = W1[0].astype(np.float16)
    w1r[1] = (W1[1].astype(np.float64) + a5 * W1[2].astype(np.float64)).astype(np.float16)
    w1r[2] = ((1.0 - a5) * W1[2].astype(np.float64)).astype(np.float16)
    w1r[3] = b1.astype(np.float16)
    # zero-padded ct=128 stationary operand: w1x[f*32+rho, r*64+h] nonzero
    # only when rho == r
    w1x_np = np.zeros((128, 32 * H), np.float16)
    for r in range(32):
        for f in range(4):
            w1x_np[f * 32 + r, r * H : (r + 1) * H] = w1r[f]

    w2c_np = np.ascontiguousarray(W2.reshape(H, 1)).astype(np.float16)

    # cell_emb passes linearly through W2: per-batch constant + b2, and with
    # n-major X columns the ps2 partition p corresponds to batch b = p % 64.
    ccb = (cell_emb[cell_idx].astype(np.float64) @ W2.astype(np.float64).reshape(H)).astype(np.float32)
    bias_p = (ccb[np.arange(128) % B] + np.float32(np.asarray(b2).reshape(-1)[0]))
    b2v_np = np.broadcast_to(bias_p[:, None], (128, NG * YC_G)).copy().astype(np.float32)

    in_maps = []
    for c in range(CORES):
        sl = slice(c * NLOC, (c + 1) * NLOC)
        Acore = A[:, sl].reshape(W, 128, WLOC, 128)[sig]      # [i, p, d, j]
        a2T_c = np.ascontiguousarray(Acore.transpose(2, 1, 0, 3)).reshape(
            WLOC, 128, W * 128
        ).astype(np.float16)

        # COO for the scatter-capable quarters, from the SAME fp16 values
        coov_c = np.zeros((NCK, 128, P_PAD), np.float16)
        cooi_c = -np.ones((NCK, 128, P_PAD), np.int16)
        for si, kq in enumerate(SCAT_KS):
            d, q = divmod(kq, NQ)
            sub = a2T_c[d][:, q * QW * 128 : (q + 1) * QW * 128].reshape(128, NCH_Q, CHUNK)
            pp, tt, cc = np.nonzero(sub)
            rows = pp * NCH_Q + tt
            cnt = np.bincount(rows, minlength=128 * NCH_Q)
            assert cnt.max() <= P_PAD, f"chunk row overflow: {cnt.max()} > {P_PAD}"
            offs = np.zeros(128 * NCH_Q, np.int64)
            np.cumsum(cnt[:-1], out=offs[1:])
            pos = np.arange(len(rows)) - offs[rows]
            for t in range(NCH_Q):
                m = tt == t
                ck = si * NCH_Q + t
                cooi_c[ck, pp[m], pos[m]] = cc[m].astype(np.int16)
                coov_c[ck, pp[m], pos[m]] = sub[pp[m], t, cc[m]]
        coov_dev = np.ascontiguousarray(coov_c.transpose(1, 0, 2)).reshape(128, NCK * P_PAD)
        cooi_dev = np.ascontiguousarray(cooi_c.transpose(1, 0, 2)).reshape(128, NCK * P_PAD)

        h0t_c = np.ascontiguousarray(
            u_pad[:, sl].reshape(B, WLOC, 128).transpose(2, 1, 0).reshape(128, WLOC * B)
        ).astype(np.float32)

        # X rows (ctl, u, -, ones) in the partition-spread layout:
        # x4bP[f*32+r, g*1280 + c*16 + t] = X_f[group g, col c*512 + r*16 + t]
        x4bP_c = np.zeros((128, NG * XW_HOST), np.float16)
        for f, row in ((0, ctl_pad[:, sl]), (1, u_pad[:, sl]), (3, None)):
            if row is None:
                flat = np.ones(NLOC * B, np.float32)
            else:
                flat = np.ascontiguousarray(row.T).reshape(-1)  # [n*B + b]
            v = flat.reshape(NG, 80, 32, 16).transpose(2, 0, 1, 3).reshape(32, NG * XW_HOST)
            x4bP_c[f * 32 : (f + 1) * 32] = v.astype(np.float16)

        in_maps.append(
            {
                "a2T": a2T_c,
                "h0sig": h0sig_np,
                "h0t": h0t_c,
                "coov": coov_dev,
                "cooi": cooi_dev,
                "x4bP": x4bP_c,
                "w1x": w1x_np,
                "w2c": w2c_np,
                "b2v": b2v_np,
                "alph": alph_np,
            }
        )
    return in_maps


XW_HOST = XCOLS // 32


def kernel(
    ctl_base,
    u_raw,
    g_logits,
    alpha_logits,
    cell_emb,
    W1,
    b1,
    W2,
    b2,
    edge_val,
    edge_src,
    edge_dst,
    cell_idx,
):
    from concourse.bass_utils import run_bass_kernel_spmd

    args = dict(
        ctl_base=np.asarray(ctl_base), u_raw=np.asarray(u_raw),
        g_logits=np.asarray(g_logits), alpha_logits=np.asarray(alpha_logits),
        cell_emb=np.asarray(cell_emb), W1=np.asarray(W1), b1=np.asarray(b1),
        W2=np.asarray(W2), b2=np.asarray(b2), edge_val=np.asarray(edge_val),
        edge_src=np.asarray(edge_src), edge_dst=np.asarray(edge_dst),
        cell_idx=np.asarray(cell_idx),
    )
    in_maps = _host_prep(**args)
    nc = _build_program()
    _CACHE["in_maps"] = in_maps
    res = run_bass_kernel_spmd(nc, in_maps, core_ids=list(range(CORES)))
    out = np.concatenate([res.results[c]["y"] for c in range(CORES)], axis=1)
    return np.ascontiguousarray(out[:, :N]).astype(np.float32)


# revision 41
# speedup vs baseline: 11675.0993x; 1.0316x over previous
"""GNN message passing (nn_OPID_78769700208710) on 8 TRN2 NeuronCores.

The 6-relation edge lists are combined on host into one sparse operator
(w[e] = sign_r * softplus(g_r) * val[e]) and materialized dense in fp16.
Propagation h_{k+1} = a_k*h0 + (1-a_k)*(h @ A) runs 6 steps on device;
decode (per-node MLP) is fused into step 6.

Sharding: destination-column model parallelism — core c owns dst columns
[c*2560, (c+1)*2560).  A's slice is stored dst-major as 20 "column panels"
[128 src x (160 win * 128 dst)] so each dst block's PSUM accumulator
completes early in the step, letting the inter-step h exchange and the
decode overlap compute.

Panel materialization is split between two engines running concurrently:
  - DMA streams most quarter-panels from HBM, and
  - GPSIMD local_scatter densifies the rest from an SBUF-resident COO copy
    of the same fp16 values (A is ~0.6% dense so the COO fits in SBUF),
    adding a second materialization pipe on an otherwise idle engine.

h exchange: each step's new h slice is AllGathered in 4 groups of 5 dst
blocks.  Window order is permuted (sigma) to group-arrival order, and deep
quarter/PSUM buffering absorbs the exchange latency at step starts.

Decode: the step-6 alpha-mix is folded into W1 on host (h0 = u_raw), so
msg6 feeds the MLP directly.  X uses a partition-spread layout
([128, 1280] per step: partition f*32+r holds X_f for columns r*16+t of
each 512-column chunk) with a zero-padded ct=128 stationary W1, so X fills
are cheap 128-partition DMAs.  cell_emb passes linearly through W2 and is
folded (with b2) into a per-partition bias.
"""

import numpy as np

N = 20000
NP = 20480          # padded nodes: 160 windows * 128
W = 160             # src windows of 128
B = 64              # batch
CORES = 8
NLOC = NP // CORES  # 2560 dst nodes per core
WLOC = NLOC // 128  # 20 dst blocks per core
H = 64
STEPS = 6
SIGNS = (1.0, -1.0, 1.0, -1.0, 1.0, -1.0)

NG = 4              # decode groups per step (xP/y granularity)
BPG = WLOC // NG    # 5 dst blocks per decode group
NG_EX = 2           # exchange groups per step (collective granularity)
BPG_EX = WLOC // NG_EX  # 10 dst blocks per exchange group
# position i in sigma-order <-> global window SIGMA[i]
SIGMA = [
    20 * p + BPG_EX * g + j
    for g in range(NG_EX)
    for p in range(CORES)
    for j in range(BPG_EX)
]

NQ = 4              # quarters per column panel
QW = W // NQ        # 40 window-positions per quarter
CHUNK = 1280        # local_scatter chunk (columns) — must be <= 2046
NCH_Q = QW * 128 // CHUNK  # 4 chunks per quarter
P_PAD = 32          # padded nonzeros per (chunk, partition-row)

# quarters (k = d*NQ + q) with COO staged in SBUF for gpsimd local_scatter.
# Steps 1-5: only blocks d%10 in {3..9} — the Pool engine is blocked ~48us by
# the AllGather right when blocks d%10 in {0,1,2} of the next group
# materialize, so those are always DMA-sourced.  Step 6 has no collectives:
# spread set, sized so DMA and gpsimd finish together.
_KS_OK = [d * NQ + q for d in range(WLOC) if d % BPG_EX >= 2 for q in range(NQ)]
SCAT_P = [k for i, k in enumerate(_KS_OK) if (i * 28) % len(_KS_OK) < 28]
SCAT_6 = [k for k in range(WLOC * NQ) if (k * 38) % (WLOC * NQ) < 38]
SCAT_KS = sorted(set(SCAT_P) | set(SCAT_6))
SCAT_RANK = {kq: i for i, kq in enumerate(SCAT_KS)}
SET_P = frozenset(SCAT_P)
SET_6 = frozenset(SCAT_6)
NCK = len(SCAT_KS) * NCH_Q  # COO chunks per core

XCOLS = BPG * 128 * B       # 40960 X columns per group
YC_G = XCOLS // 128         # 320 y (ps2) columns per group

_CACHE = {}

import os as _os
_NO_SCAT = _os.environ.get("NO_SCAT", "0") == "1"


def _np_softplus(x):
    return np.log1p(np.exp(-np.abs(x))) + np.maximum(x, 0.0)


def _np_sigmoid(x):
    return 1.0 / (1.0 + np.exp(-x))


def _build_program(compile_=True, debug=False):
    key = ("nc2", debug)
    if key in _CACHE:
        return _CACHE[key]

    import concourse.bacc as bacc
    import concourse.mybir as mybir
    from concourse import tile

    f16 = mybir.dt.float16
    f32 = mybir.dt.float32
    i16 = mybir.dt.int16
    AF = mybir.ActivationFunctionType
    OP = mybir.AluOpType

    nc = bacc.Bacc(
        "TRN2",
        target_bir_lowering=False,
        debug=False,
        enable_asserts=False,
        num_devices=CORES,
    )

    a2T = nc.dram_tensor("a2T", [WLOC, 128, W * 128], f16, kind="ExternalInput")
    h0sig = nc.dram_tensor("h0sig", [128, W * B], f16, kind="ExternalInput")
    h0t = nc.dram_tensor("h0t", [128, WLOC * B], f32, kind="ExternalInput")
    coov = nc.dram_tensor("coov", [128, NCK * P_PAD], f16, kind="ExternalInput")
    cooi = nc.dram_tensor("cooi", [128, NCK * P_PAD], i16, kind="ExternalInput")
    x4bP = nc.dram_tensor("x4bP", [128, NG * XCOLS // 32], f16, kind="ExternalInput")
    w1x = nc.dram_tensor("w1x", [128, 32 * H], f16, kind="ExternalInput")
    w2c = nc.dram_tensor("w2c", [H, 1], f16, kind="ExternalInput")
    b2v = nc.dram_tensor("b2v", [128, NG * YC_G], f32, kind="ExternalInput")
    alph = nc.dram_tensor("alph", [128, 2 * STEPS], f32, kind="ExternalInput")
    y = nc.dram_tensor("y", [B, NLOC], f32, kind="ExternalOutput")
    if debug:
        dbg_h = nc.dram_tensor("dbg_h", [STEPS - 1, 128, W * B], f16, kind="ExternalOutput")
        dbg_x = nc.dram_tensor("dbg_x", [128, NG * XCOLS // 32], f16, kind="ExternalOutput")

    XW = XCOLS // 32            # 1280 xP columns per group

    with tile.TileContext(nc) as tc:
        with (
            tc.tile_pool(name="const", bufs=1) as constp,
            tc.tile_pool(name="qp", bufs=11) as qpool,
            tc.tile_pool(name="tmp", bufs=2) as tmpp,
            tc.tile_pool(name="big", bufs=2) as bip,
            tc.tile_pool(name="hds", bufs=2) as hdsp,
            tc.tile_pool(name="ysb", bufs=2) as ysbp,
            tc.tile_pool(name="accps", bufs=5, space="PSUM") as accp,
            tc.tile_pool(name="psa", bufs=2, space="PSUM") as psap,
            tc.tile_pool(name="ps2", bufs=1, space="PSUM") as ps2p,
            tc.tile_pool(name="dram", bufs=2, space="DRAM") as dramp,
        ):
            # ---- persistent SBUF state ----
            # double-buffered by step parity: step k reads set k%2, the
            # in-step exchange writes set (k+1)%2 for the next step
            h_slab = [
                [
                    constp.tile([128, QW * B], f16, tag=f"hsl{s}{g}", name=f"hsl{s}{g}")
                    for g in range(NG)
                ]
                for s in range(2)
            ]
            h0t_sb = constp.tile([128, WLOC * B], f32, tag="h0t")
            coov_sb = constp.tile([128, NCK * P_PAD], f16, tag="coov")
            cooi_sb = constp.tile([128, NCK * P_PAD], i16, tag="cooi")
            alph_sb = constp.tile([128, 2 * STEPS], f32, tag="alph")
            w1x_sb = constp.tile([128, 32 * H], f16, tag="w1x")
            w2c_sb = constp.tile([H, 1], f16, tag="w2c")
            b2v_sb = constp.tile([128, NG * YC_G], f32, tag="b2v")
            xP = constp.tile([128, NG * XW], f16, tag="xP")

            for g in range(NG):
                nc.sync.dma_start(
                    h_slab[0][g][:], h0sig.ap()[:, g * QW * B : (g + 1) * QW * B]
                )
            nc.sync.dma_start(h0t_sb[:], h0t.ap())
            nc.sync.dma_start(coov_sb[:], coov.ap())
            nc.sync.dma_start(cooi_sb[:], cooi.ap())
            nc.sync.dma_start(alph_sb[:], alph.ap())
            nc.sync.dma_start(w1x_sb[:], w1x.ap())
            nc.sync.dma_start(w2c_sb[:], w2c.ap())
            nc.sync.dma_start(b2v_sb[:], b2v.ap())
            nc.sync.dma_start(xP[:], x4bP.ap())

            # DRAM bounce for the step-6 msg -> X row reshuffle
            xrow2 = dramp.tile([WLOC, 128, B], f16, tag="xrow2", bufs=1)

            for k in range(STEPS):
                for d in range(WLOC):
                    g = d // BPG
                    acc = accp.tile([128, B], f32, tag="acc")
                    for q in range(NQ):
                        kq = d * NQ + q
                        qt = qpool.tile([128, QW * 128], f16, tag="qp")
                        use_scat = (not _NO_SCAT) and kq in (
                            SET_6 if k == STEPS - 1 else SET_P
                        )
                        if use_scat:
                            ck0 = SCAT_RANK[kq] * NCH_Q
                            for t in range(NCH_Q):
                                ck = ck0 + t
                                nc.gpsimd.local_scatter(
                                    qt[:, t * CHUNK : (t + 1) * CHUNK],
                                    coov_sb[:, ck * P_PAD : (ck + 1) * P_PAD],
                                    cooi_sb[:, ck * P_PAD : (ck + 1) * P_PAD],
                                    128,
                                    CHUNK,
                                    P_PAD,
                                )
                        else:
                            nc.sync.dma_start(
                                qt[:], a2T.ap()[d][:, q * QW * 128 : (q + 1) * QW * 128]
                            )
                        for i in range(QW):
                            nc.tensor.matmul(
                                acc[:],
                                lhsT=qt[:, i * 128 : (i + 1) * 128],
                                rhs=h_slab[k % 2][q][:, i * B : (i + 1) * B],
                                start=(q == 0 and i == 0),
                                stop=(q == NQ - 1 and i == QW - 1),
                            )

                    if k < STEPS - 1:
                        # epilogue: h_new = a*h0 + (1-a)*msg, fp16
                        if d % BPG_EX == 0:
                            bi_sb = bip.tile([128, BPG_EX * B], f16, tag="bi")
                        h0a = tmpp.tile([128, B], f32, tag="h0a")
                        nc.scalar.activation(
                            h0a[:],
                            h0t_sb[:, d * B : (d + 1) * B],
                            AF.Copy,
                            scale=alph_sb[:, k : k + 1],
                        )
                        nc.vector.scalar_tensor_tensor(
                            bi_sb[:, (d % BPG_EX) * B : (d % BPG_EX + 1) * B],
                            acc[:],
                            alph_sb[:, STEPS + k : STEPS + k + 1],
                            h0a[:],
                            OP.mult,
                            OP.add,
                        )
                        if d % BPG_EX == BPG_EX - 1:
                            ge = d // BPG_EX
                            bi_d = dramp.tile([128, BPG_EX * B], f16, tag="bi_d")
                            bo_d = dramp.tile(
                                [CORES, 128, BPG_EX * B], f16, tag="bo_d"
                            )
                            nc.sync.dma_start(bi_d[:], bi_sb[:])
                            nc.gpsimd.collective_compute(
                                "AllGather",
                                OP.bypass,
                                replica_groups=[list(range(CORES))],
                                ins=[bi_d.opt()],
                                outs=[bo_d.opt()],
                            )
                            # group ge covers positions [80*ge, 80*ge+80) =
                            # slabs 2*ge (cores 0-3) and 2*ge+1 (cores 4-7)
                            for half in range(2):
                                slab = h_slab[(k + 1) % 2][2 * ge + half]
                                nc.sync.dma_start(
                                    slab[:].rearrange("p (c f) -> p c f", c=4),
                                    bo_d[:][4 * half : 4 * half + 4].rearrange(
                                        "c p f -> p c f"
                                    ),
                                )
                                if debug:
                                    nc.sync.dma_start(
                                        dbg_h.ap()[k][
                                            :,
                                            (2 * ge + half)
                                            * QW
                                            * B : (2 * ge + half + 1)
                                            * QW
                                            * B,
                                        ],
                                        slab[:],
                                    )
                    else:
                        # step 6: stage raw msg6 (alpha folded into W1) to
                        # DRAM, then decode this block's 16 X-chunks right
                        # away so only the last block's decode is a tail
                        dl = d % BPG
                        if dl == 0:
                            ps2 = ps2p.tile([128, YC_G], f32, tag="ps2")
                        st16 = tmpp.tile([128, B], f16, tag="st16")
                        nc.scalar.activation(st16[:], acc[:], AF.Copy)
                        nc.sync.dma_start(xrow2[:][d], st16[:])
                        # scatter this block's msg6 into xP (f=2 partitions)
                        nc.sync.dma_start(
                            xP[
                                64:96, g * XW + dl * 256 : g * XW + (dl + 1) * 256
                            ].rearrange("r (c t) -> r c t", t=16),
                            xrow2[:]
                            .rearrange("d p f -> (d p f)")[
                                g * XCOLS + dl * 8192 : g * XCOLS + (dl + 1) * 8192
                            ]
                            .rearrange("(c r t) -> r c t", r=32, t=16),
                        )
                        for c8 in range(dl * 16, (dl + 1) * 16):
                            psA = psap.tile([H, 512], f32, tag="psA")
                            rhs = xP[:, g * XW + c8 * 16 : g * XW + (c8 + 1) * 16]
                            for r in range(32):
                                nc.tensor.matmul(
                                    psA[:, r * 16 : (r + 1) * 16],
                                    lhsT=w1x_sb[:, r * H : (r + 1) * H],
                                    rhs=rhs,
                                    start=True,
                                    stop=True,
                                )
                            hds = hdsp.tile([H, 512], f16, tag="hds")
                            nc.scalar.activation(hds[:], psA[:], AF.Relu)
                            for i in range(4):
                                col = c8 * 4 + i
                                nc.tensor.matmul(
                                    ps2[:, col : col + 1],
                                    lhsT=hds[:, i * 128 : (i + 1) * 128],
                                    rhs=w2c_sb[:],
                                    start=True,
                                    stop=True,
                                )

                        if d % BPG == BPG - 1:
                            ysb = ysbp.tile([128, YC_G], f32, tag="ysb")
                            nc.vector.scalar_tensor_tensor(
                                ysb[:],
                                ps2[:],
                                1.0,
                                b2v_sb[:, g * YC_G : (g + 1) * YC_G],
                                OP.mult,
                                OP.add,
                            )
                            if debug:
                                nc.sync.dma_start(
                                    dbg_x.ap()[:, g * XW : (g + 1) * XW],
                                    xP[:, g * XW : (g + 1) * XW],
                                )
                            ysp = y.ap().rearrange("b (f t) -> f t b", t=2)
                            for t in range(2):
                                nc.sync.dma_start(
                                    ysp[g * YC_G : (g + 1) * YC_G][:, t, :].rearrange(
                                        "f b -> b f"
                                    ),
                                    ysb[t * 64 : (t + 1) * 64, :],
                                )

    if compile_:
        nc.compile()
    _CACHE[key] = nc
    return nc


def _host_prep(ctl_base, u_raw, g_logits, alpha_logits, cell_emb,
               W1, b1, W2, b2, edge_val, edge_src, edge_dst, cell_idx):
    g = _np_softplus(np.asarray(g_logits, np.float64))
    alphas = _np_sigmoid(np.asarray(alpha_logits, np.float64))

    A = np.zeros((NP, NP), np.float32)
    for r in range(6):
        w = (SIGNS[r] * g[r]) * np.asarray(edge_val[r], np.float64)
        np.add.at(A, (edge_src[r], edge_dst[r]), w.astype(np.float32))

    u_pad = np.zeros((B, NP), np.float32)
    u_pad[:, :N] = u_raw
    ctl_pad = np.zeros((B, NP), np.float32)
    ctl_pad[:, :N] = ctl_base

    sig = np.asarray(SIGMA)
    # h0 in sigma window layout: [p, i*B+b] = u[b, SIGMA[i]*128+p]
    h0sig_np = np.ascontiguousarray(
        u_pad.reshape(B, W, 128)[:, sig, :].transpose(2, 1, 0).reshape(128, W * B)
    ).astype(np.float16)

    alph_np = np.zeros((128, 2 * STEPS), np.float32)
    alph_np[:, :STEPS] = alphas.astype(np.float32)
    alph_np[:, STEPS:] = (1.0 - alphas).astype(np.float32)

    a5 = alphas[STEPS - 1]
    w1r = np.zeros((4, H), np.float16)
    w1r[0] = W1[0].astype(np.float16)
    w1r[1] = (W1[1].astype(np.float64) + a5 * W1[2].astype(np.float64)).astype(np.float16)
    w1r[2] = ((1.0 - a5) * W1[2].astype(np.float64)).astype(np.float16)
    w1r[3] = b1.astype(np.float16)
    # zero-padded ct=128 stationary operand: w1x[f*32+rho, r*64+h] nonzero
    # only when rho == r
    w1x_np = np.zeros((128, 32 * H), np.float16)
    for r in range(32):
        for f in range(4):
            w1x_np[f * 32 + r, r * H : (r + 1) * H] = w1r[f]

    w2c_np = np.ascontiguousarray(W2.reshape(H, 1)).astype(np.float16)

    # cell_emb passes linearly through W2: per-batch constant + b2, and with
    # n-major X columns the ps2 partition p corresponds to batch b = p % 64.
    ccb = (cell_emb[cell_idx].astype(np.float64) @ W2.astype(np.float64).reshape(H)).astype(np.float32)
    bias_p = (ccb[np.arange(128) % B] + np.float32(np.asarray(b2).reshape(-1)[0]))
    b2v_np = np.broadcast_to(bias_p[:, None], (128, NG * YC_G)).copy().astype(np.float32)

    in_maps = []
    for c in range(CORES):
        sl = slice(c * NLOC, (c + 1) * NLOC)
        Acore = A[:, sl].reshape(W, 128, WLOC, 128)[sig]      # [i, p, d, j]
        a2T_c = np.ascontiguousarray(Acore.transpose(2, 1, 0, 3)).reshape(
            WLOC, 128, W * 128
        ).astype(np.float16)

        # COO for the scatter-capable quarters, from the SAME fp16 values
        coov_c = np.zeros((NCK, 128, P_PAD), np.float16)
        cooi_c = -np.ones((NCK, 128, P_PAD), np.int16)
        for si, kq in enumerate(SCAT_KS):
            d, q = divmod(kq, NQ)
            sub = a2T_c[d][:, q * QW * 128 : (q + 1) * QW * 128].reshape(128, NCH_Q, CHUNK)
            pp, tt, cc = np.nonzero(sub)
            rows = pp * NCH_Q + tt
            cnt = np.bincount(rows, minlength=128 * NCH_Q)
            assert cnt.max() <= P_PAD, f"chunk row overflow: {cnt.max()} > {P_PAD}"
            offs = np.zeros(128 * NCH_Q, np.int64)
            np.cumsum(cnt[:-1], out=offs[1:])
            pos = np.arange(len(rows)) - offs[rows]
            for t in range(NCH_Q):
                m = tt == t
                ck = si * NCH_Q + t
                cooi_c[ck, pp[m], pos[m]] = cc[m].astype(np.int16)
                coov_c[ck, pp[m], pos[m]] = sub[pp[m], t, cc[m]]
        coov_dev = np.ascontiguousarray(coov_c.transpose(1, 0, 2)).reshape(128, NCK * P_PAD)
        cooi_dev = np.ascontiguousarray(cooi_c.transpose(1, 0, 2)).reshape(128, NCK * P_PAD)

        h0t_c = np.ascontiguousarray(
            u_pad[:, sl].reshape(B, WLOC, 128).transpose(2, 1, 0).reshape(128, WLOC * B)
        ).astype(np.float32)

        # X rows (ctl, u, -, ones) in the partition-spread layout:
        # x4bP[f*32+r, g*1280 + c*16 + t] = X_f[group g, col c*512 + r*16 + t]
        x4bP_c = np.zeros((128, NG * XW_HOST), np.float16)
        for f, row in ((0, ctl_pad[:, sl]), (1, u_pad[:, sl]), (3, None)):
            if row is None:
                flat = np.ones(NLOC * B, np.float32)
            else:
                flat = np.ascontiguousarray(row.T).reshape(-1)  # [n*B + b]
            v = flat.reshape(NG, 80, 32, 16).transpose(2, 0, 1, 3).reshape(32, NG * XW_HOST)
            x4bP_c[f * 32 : (f + 1) * 32] = v.astype(np.float16)

        in_maps.append(
            {
                "a2T": a2T_c,
                "h0sig": h0sig_np,
                "h0t": h0t_c,
                "coov": coov_dev,
                "cooi": cooi_dev,
                "x4bP": x4bP_c,
                "w1x": w1x_np,
                "w2c": w2c_np,
                "b2v": b2v_np,
                "alph": alph_np,
            }
        )
    return in_maps


XW_HOST = XCOLS // 32


def kernel(
    ctl_base,
    u_raw,
    g_logits,
    alpha_logits,
    cell_emb,
    W1,
    b1,
    W2,
    b2,
    edge_val,
    edge_src,
    edge_dst,
    cell_idx,
):
    from concourse.bass_utils import run_bass_kernel_spmd

    args = dict(
        ctl_base=np.asarray(ctl_base), u_raw=np.asarray(u_raw),
        g_logits=np.asarray(g_logits), alpha_logits=np.asarray(alpha_logits),
        cell_emb=np.asarray(cell_emb), W1=np.asarray(W1), b1=np.asarray(b1),
        W2=np.asarray(W2), b2=np.asarray(b2), edge_val=np.asarray(edge_val),
        edge_src=np.asarray(edge_src), edge_dst=np.asarray(edge_dst),
        cell_idx=np.asarray(cell_idx),
    )
    in_maps = _host_prep(**args)
    nc = _build_program()
    _CACHE["in_maps"] = in_maps
    res = run_bass_kernel_spmd(nc, in_maps, core_ids=list(range(CORES)))
    out = np.concatenate([res.results[c]["y"] for c in range(CORES)], axis=1)
    return np.ascontiguousarray(out[:, :N]).astype(np.float32)


# revision 42
# speedup vs baseline: 12101.2211x; 1.0365x over previous
"""GNN message passing (nn_OPID_78769700208710) on 8 TRN2 NeuronCores.

The 6-relation edge lists are combined on host into one sparse operator
(w[e] = sign_r * softplus(g_r) * val[e]) and materialized dense in fp16.
Propagation h_{k+1} = a_k*h0 + (1-a_k)*(h @ A) runs 6 steps on device;
decode (per-node MLP) is fused into step 6.

Sharding: destination-column model parallelism — core c owns dst columns
[c*2560, (c+1)*2560).  A's slice is stored dst-major as 20 "column panels"
[128 src x (160 win * 128 dst)] so each dst block's PSUM accumulator
completes early in the step, letting the inter-step h exchange and the
decode overlap compute.

Panel materialization is split between two engines running concurrently:
  - DMA streams most quarter-panels from HBM, and
  - GPSIMD local_scatter densifies the rest from an SBUF-resident COO copy
    of the same fp16 values (A is ~0.6% dense so the COO fits in SBUF),
    adding a second materialization pipe on an otherwise idle engine.

h exchange: each step's new h slice is AllGathered in 4 groups of 5 dst
blocks.  Window order is permuted (sigma) to group-arrival order, and deep
quarter/PSUM buffering absorbs the exchange latency at step starts.

Decode: the step-6 alpha-mix is folded into W1 on host (h0 = u_raw), so
msg6 feeds the MLP directly.  X uses a partition-spread layout
([128, 1280] per step: partition f*32+r holds X_f for columns r*16+t of
each 512-column chunk) with a zero-padded ct=128 stationary W1, so X fills
are cheap 128-partition DMAs.  cell_emb passes linearly through W2 and is
folded (with b2) into a per-partition bias.
"""

import numpy as np

N = 20000
NP = 20480          # padded nodes: 160 windows * 128
W = 160             # src windows of 128
B = 64              # batch
CORES = 8
NLOC = NP // CORES  # 2560 dst nodes per core
WLOC = NLOC // 128  # 20 dst blocks per core
H = 64
STEPS = 6
SIGNS = (1.0, -1.0, 1.0, -1.0, 1.0, -1.0)

NG = 4              # decode groups per step (xP/y granularity)
BPG = WLOC // NG    # 5 dst blocks per decode group
NG_EX = 2           # exchange groups per step (collective granularity)
BPG_EX = WLOC // NG_EX  # 10 dst blocks per exchange group
# position i in sigma-order <-> global window SIGMA[i]
SIGMA = [
    20 * p + BPG_EX * g + j
    for g in range(NG_EX)
    for p in range(CORES)
    for j in range(BPG_EX)
]

NQ = 4              # quarters per column panel
QW = W // NQ        # 40 window-positions per quarter
CHUNK = 1280        # local_scatter chunk (columns) — must be <= 2046
NCH_Q = QW * 128 // CHUNK  # 4 chunks per quarter
P_PAD = 32          # padded nonzeros per (chunk, partition-row)

# quarters (k = d*NQ + q) with COO staged in SBUF for gpsimd local_scatter.
# Steps 1-5: only blocks d%10 in {3..9} — the Pool engine is blocked ~48us by
# the AllGather right when blocks d%10 in {0,1,2} of the next group
# materialize, so those are always DMA-sourced.  Step 6 has no collectives:
# spread set, sized so DMA and gpsimd finish together.
_KS_OK = [d * NQ + q for d in range(WLOC) if d % BPG_EX >= 2 for q in range(NQ)]
SCAT_P = [k for i, k in enumerate(_KS_OK) if (i * 28) % len(_KS_OK) < 28]
SCAT_6 = [k for k in range(WLOC * NQ) if (k * 40) % (WLOC * NQ) < 40]
SCAT_KS = sorted(set(SCAT_P) | set(SCAT_6))
SCAT_RANK = {kq: i for i, kq in enumerate(SCAT_KS)}
SET_P = frozenset(SCAT_P)
SET_6 = frozenset(SCAT_6)
NCK = len(SCAT_KS) * NCH_Q  # COO chunks per core

XCOLS = BPG * 128 * B       # 40960 X columns per group
YC_G = XCOLS // 128         # 320 y (ps2) columns per group

_CACHE = {}

import os as _os
_NO_SCAT = _os.environ.get("NO_SCAT", "0") == "1"


def _np_softplus(x):
    return np.log1p(np.exp(-np.abs(x))) + np.maximum(x, 0.0)


def _np_sigmoid(x):
    return 1.0 / (1.0 + np.exp(-x))


def _build_program(compile_=True, debug=False):
    key = ("nc2", debug)
    if key in _CACHE:
        return _CACHE[key]

    import concourse.bacc as bacc
    import concourse.mybir as mybir
    from concourse import tile

    f16 = mybir.dt.float16
    f32 = mybir.dt.float32
    i16 = mybir.dt.int16
    AF = mybir.ActivationFunctionType
    OP = mybir.AluOpType

    nc = bacc.Bacc(
        "TRN2",
        target_bir_lowering=False,
        debug=False,
        enable_asserts=False,
        num_devices=CORES,
    )

    a2T = nc.dram_tensor("a2T", [WLOC, 128, W * 128], f16, kind="ExternalInput")
    h0sig = nc.dram_tensor("h0sig", [128, W * B], f16, kind="ExternalInput")
    h0t = nc.dram_tensor("h0t", [128, WLOC * B], f32, kind="ExternalInput")
    coov = nc.dram_tensor("coov", [128, NCK * P_PAD], f16, kind="ExternalInput")
    cooi = nc.dram_tensor("cooi", [128, NCK * P_PAD], i16, kind="ExternalInput")
    x4bP = nc.dram_tensor("x4bP", [128, NG * XCOLS // 32], f16, kind="ExternalInput")
    w1x = nc.dram_tensor("w1x", [128, 32 * H], f16, kind="ExternalInput")
    w2c = nc.dram_tensor("w2c", [H, 1], f16, kind="ExternalInput")
    b2v = nc.dram_tensor("b2v", [128, NG * YC_G], f32, kind="ExternalInput")
    alph = nc.dram_tensor("alph", [128, 2 * STEPS], f32, kind="ExternalInput")
    y = nc.dram_tensor("y", [B, NLOC], f32, kind="ExternalOutput")
    if debug:
        dbg_h = nc.dram_tensor("dbg_h", [STEPS - 1, 128, W * B], f16, kind="ExternalOutput")
        dbg_x = nc.dram_tensor("dbg_x", [128, NG * XCOLS // 32], f16, kind="ExternalOutput")

    XW = XCOLS // 32            # 1280 xP columns per group

    with tile.TileContext(nc) as tc:
        with (
            tc.tile_pool(name="const", bufs=1) as constp,
            tc.tile_pool(name="qp", bufs=11) as qpool,
            tc.tile_pool(name="tmp", bufs=2) as tmpp,
            tc.tile_pool(name="big", bufs=2) as bip,
            tc.tile_pool(name="hds", bufs=2) as hdsp,
            tc.tile_pool(name="ysb", bufs=2) as ysbp,
            tc.tile_pool(name="accps", bufs=5, space="PSUM") as accp,
            tc.tile_pool(name="psa", bufs=2, space="PSUM") as psap,
            tc.tile_pool(name="ps2", bufs=1, space="PSUM") as ps2p,
            tc.tile_pool(name="dram", bufs=2, space="DRAM") as dramp,
        ):
            # ---- persistent SBUF state ----
            # double-buffered by step parity: step k reads set k%2, the
            # in-step exchange writes set (k+1)%2 for the next step
            h_slab = [
                [
                    constp.tile([128, QW * B], f16, tag=f"hsl{s}{g}", name=f"hsl{s}{g}")
                    for g in range(NG)
                ]
                for s in range(2)
            ]
            h0t_sb = constp.tile([128, WLOC * B], f32, tag="h0t")
            coov_sb = constp.tile([128, NCK * P_PAD], f16, tag="coov")
            cooi_sb = constp.tile([128, NCK * P_PAD], i16, tag="cooi")
            alph_sb = constp.tile([128, 2 * STEPS], f32, tag="alph")
            w1x_sb = constp.tile([128, 32 * H], f16, tag="w1x")
            w2c_sb = constp.tile([H, 1], f16, tag="w2c")
            b2v_sb = constp.tile([128, NG * YC_G], f32, tag="b2v")
            xP = constp.tile([128, NG * XW], f16, tag="xP")

            for g in range(NG):
                nc.sync.dma_start(
                    h_slab[0][g][:], h0sig.ap()[:, g * QW * B : (g + 1) * QW * B]
                )
            nc.sync.dma_start(h0t_sb[:], h0t.ap())
            nc.sync.dma_start(coov_sb[:], coov.ap())
            nc.sync.dma_start(cooi_sb[:], cooi.ap())
            nc.sync.dma_start(alph_sb[:], alph.ap())
            nc.sync.dma_start(w1x_sb[:], w1x.ap())
            nc.sync.dma_start(w2c_sb[:], w2c.ap())
            nc.sync.dma_start(b2v_sb[:], b2v.ap())
            nc.sync.dma_start(xP[:], x4bP.ap())

            # DRAM bounce for the step-6 msg -> X row reshuffle
            xrow2 = dramp.tile([WLOC, 128, B], f16, tag="xrow2", bufs=1)

            for k in range(STEPS):
                for d in range(WLOC):
                    g = d // BPG
                    acc = accp.tile([128, B], f32, tag="acc")
                    for q in range(NQ):
                        kq = d * NQ + q
                        qt = qpool.tile([128, QW * 128], f16, tag="qp")
                        use_scat = (not _NO_SCAT) and kq in (
                            SET_6 if k == STEPS - 1 else SET_P
                        )
                        if use_scat:
                            ck0 = SCAT_RANK[kq] * NCH_Q
                            for t in range(NCH_Q):
                                ck = ck0 + t
                                nc.gpsimd.local_scatter(
                                    qt[:, t * CHUNK : (t + 1) * CHUNK],
                                    coov_sb[:, ck * P_PAD : (ck + 1) * P_PAD],
                                    cooi_sb[:, ck * P_PAD : (ck + 1) * P_PAD],
                                    128,
                                    CHUNK,
                                    P_PAD,
                                )
                        else:
                            nc.sync.dma_start(
                                qt[:], a2T.ap()[d][:, q * QW * 128 : (q + 1) * QW * 128]
                            )
                        for i in range(QW):
                            nc.tensor.matmul(
                                acc[:],
                                lhsT=qt[:, i * 128 : (i + 1) * 128],
                                rhs=h_slab[k % 2][q][:, i * B : (i + 1) * B],
                                start=(q == 0 and i == 0),
                                stop=(q == NQ - 1 and i == QW - 1),
                            )

                    if k < STEPS - 1:
                        # epilogue: h_new = a*h0 + (1-a)*msg, fp16
                        if d % BPG_EX == 0:
                            bi_sb = bip.tile([128, BPG_EX * B], f16, tag="bi")
                        h0a = tmpp.tile([128, B], f32, tag="h0a")
                        nc.scalar.activation(
                            h0a[:],
                            h0t_sb[:, d * B : (d + 1) * B],
                            AF.Copy,
                            scale=alph_sb[:, k : k + 1],
                        )
                        nc.vector.scalar_tensor_tensor(
                            bi_sb[:, (d % BPG_EX) * B : (d % BPG_EX + 1) * B],
                            acc[:],
                            alph_sb[:, STEPS + k : STEPS + k + 1],
                            h0a[:],
                            OP.mult,
                            OP.add,
                        )
                        if d % BPG_EX == BPG_EX - 1:
                            ge = d // BPG_EX
                            bi_d = dramp.tile([128, BPG_EX * B], f16, tag="bi_d")
                            bo_d = dramp.tile(
                                [CORES, 128, BPG_EX * B], f16, tag="bo_d"
                            )
                            nc.sync.dma_start(bi_d[:], bi_sb[:])
                            nc.gpsimd.collective_compute(
                                "AllGather",
                                OP.bypass,
                                replica_groups=[list(range(CORES))],
                                ins=[bi_d.opt()],
                                outs=[bo_d.opt()],
                            )
                            # group ge covers positions [80*ge, 80*ge+80) =
                            # slabs 2*ge (cores 0-3) and 2*ge+1 (cores 4-7)
                            for half in range(2):
                                slab = h_slab[(k + 1) % 2][2 * ge + half]
                                nc.sync.dma_start(
                                    slab[:].rearrange("p (c f) -> p c f", c=4),
                                    bo_d[:][4 * half : 4 * half + 4].rearrange(
                                        "c p f -> p c f"
                                    ),
                                )
                                if debug:
                                    nc.sync.dma_start(
                                        dbg_h.ap()[k][
                                            :,
                                            (2 * ge + half)
                                            * QW
                                            * B : (2 * ge + half + 1)
                                            * QW
                                            * B,
                                        ],
                                        slab[:],
                                    )
                    else:
                        # step 6: stage raw msg6 (alpha folded into W1) to
                        # DRAM, then decode this block's 16 X-chunks right
                        # away so only the last block's decode is a tail
                        dl = d % BPG
                        if dl == 0:
                            ps2 = ps2p.tile([128, YC_G], f32, tag="ps2")
                        st16 = tmpp.tile([128, B], f16, tag="st16")
                        nc.scalar.activation(st16[:], acc[:], AF.Copy)
                        nc.sync.dma_start(xrow2[:][d], st16[:])
                        # scatter this block's msg6 into xP (f=2 partitions)
                        nc.sync.dma_start(
                            xP[
                                64:96, g * XW + dl * 256 : g * XW + (dl + 1) * 256
                            ].rearrange("r (c t) -> r c t", t=16),
                            xrow2[:]
                            .rearrange("d p f -> (d p f)")[
                                g * XCOLS + dl * 8192 : g * XCOLS + (dl + 1) * 8192
                            ]
                            .rearrange("(c r t) -> r c t", r=32, t=16),
                        )
                        for c8 in range(dl * 16, (dl + 1) * 16):
                            psA = psap.tile([H, 512], f32, tag="psA")
                            rhs = xP[:, g * XW + c8 * 16 : g * XW + (c8 + 1) * 16]
                            for r in range(32):
                                nc.tensor.matmul(
                                    psA[:, r * 16 : (r + 1) * 16],
                                    lhsT=w1x_sb[:, r * H : (r + 1) * H],
                                    rhs=rhs,
                                    start=True,
                                    stop=True,
                                )
                            hds = hdsp.tile([H, 512], f16, tag="hds")
                            if c8 % 2 == 0:
                                nc.scalar.activation(hds[:], psA[:], AF.Relu)
                            else:
                                nc.vector.tensor_scalar_max(hds[:], psA[:], 0.0)
                            for i in range(4):
                                col = c8 * 4 + i
                                nc.tensor.matmul(
                                    ps2[:, col : col + 1],
                                    lhsT=hds[:, i * 128 : (i + 1) * 128],
                                    rhs=w2c_sb[:],
                                    start=True,
                                    stop=True,
                                )

                        if d % BPG == BPG - 1:
                            ysb = ysbp.tile([128, YC_G], f32, tag="ysb")
                            nc.vector.scalar_tensor_tensor(
                                ysb[:],
                                ps2[:],
                                1.0,
                                b2v_sb[:, g * YC_G : (g + 1) * YC_G],
                                OP.mult,
                                OP.add,
                            )
                            if debug:
                                nc.sync.dma_start(
                                    dbg_x.ap()[:, g * XW : (g + 1) * XW],
                                    xP[:, g * XW : (g + 1) * XW],
                                )
                            ysp = y.ap().rearrange("b (f t) -> f t b", t=2)
                            for t in range(2):
                                nc.sync.dma_start(
                                    ysp[g * YC_G : (g + 1) * YC_G][:, t, :].rearrange(
                                        "f b -> b f"
                                    ),
                                    ysb[t * 64 : (t + 1) * 64, :],
                                )

    if compile_:
        nc.compile()
    _CACHE[key] = nc
    return nc


def _host_prep(ctl_base, u_raw, g_logits, alpha_logits, cell_emb,
               W1, b1, W2, b2, edge_val, edge_src, edge_dst, cell_idx):
    g = _np_softplus(np.asarray(g_logits, np.float64))
    alphas = _np_sigmoid(np.asarray(alpha_logits, np.float64))

    A = np.zeros((NP, NP), np.float32)
    for r in range(6):
        w = (SIGNS[r] * g[r]) * np.asarray(edge_val[r], np.float64)
        np.add.at(A, (edge_src[r], edge_dst[r]), w.astype(np.float32))

    u_pad = np.zeros((B, NP), np.float32)
    u_pad[:, :N] = u_raw
    ctl_pad = np.zeros((B, NP), np.float32)
    ctl_pad[:, :N] = ctl_base

    sig = np.asarray(SIGMA)
    # h0 in sigma window layout: [p, i*B+b] = u[b, SIGMA[i]*128+p]
    h0sig_np = np.ascontiguousarray(
        u_pad.reshape(B, W, 128)[:, sig, :].transpose(2, 1, 0).reshape(128, W * B)
    ).astype(np.float16)

    alph_np = np.zeros((128, 2 * STEPS), np.float32)
    alph_np[:, :STEPS] = alphas.astype(np.float32)
    alph_np[:, STEPS:] = (1.0 - alphas).astype(np.float32)

    a5 = alphas[STEPS - 1]
    w1r = np.zeros((4, H), np.float16)
    w1r[0] = W1[0].astype(np.float16)
    w1r[1] = (W1[1].astype(np.float64) + a5 * W1[2].astype(np.float64)).astype(np.float16)
    w1r[2] = ((1.0 - a5) * W1[2].astype(np.float64)).astype(np.float16)
    w1r[3] = b1.astype(np.float16)
    # zero-padded ct=128 stationary operand: w1x[f*32+rho, r*64+h] nonzero
    # only when rho == r
    w1x_np = np.zeros((128, 32 * H), np.float16)
    for r in range(32):
        for f in range(4):
            w1x_np[f * 32 + r, r * H : (r + 1) * H] = w1r[f]

    w2c_np = np.ascontiguousarray(W2.reshape(H, 1)).astype(np.float16)

    # cell_emb passes linearly through W2: per-batch constant + b2, and with
    # n-major X columns the ps2 partition p corresponds to batch b = p % 64.
    ccb = (cell_emb[cell_idx].astype(np.float64) @ W2.astype(np.float64).reshape(H)).astype(np.float32)
    bias_p = (ccb[np.arange(128) % B] + np.float32(np.asarray(b2).reshape(-1)[0]))
    b2v_np = np.broadcast_to(bias_p[:, None], (128, NG * YC_G)).copy().astype(np.float32)

    in_maps = []
    for c in range(CORES):
        sl = slice(c * NLOC, (c + 1) * NLOC)
        Acore = A[:, sl].reshape(W, 128, WLOC, 128)[sig]      # [i, p, d, j]
        a2T_c = np.ascontiguousarray(Acore.transpose(2, 1, 0, 3)).reshape(
            WLOC, 128, W * 128
        ).astype(np.float16)

        # COO for the scatter-capable quarters, from the SAME fp16 values
        coov_c = np.zeros((NCK, 128, P_PAD), np.float16)
        cooi_c = -np.ones((NCK, 128, P_PAD), np.int16)
        for si, kq in enumerate(SCAT_KS):
            d, q = divmod(kq, NQ)
            sub = a2T_c[d][:, q * QW * 128 : (q + 1) * QW * 128].reshape(128, NCH_Q, CHUNK)
            pp, tt, cc = np.nonzero(sub)
            rows = pp * NCH_Q + tt
            cnt = np.bincount(rows, minlength=128 * NCH_Q)
            assert cnt.max() <= P_PAD, f"chunk row overflow: {cnt.max()} > {P_PAD}"
            offs = np.zeros(128 * NCH_Q, np.int64)
            np.cumsum(cnt[:-1], out=offs[1:])
            pos = np.arange(len(rows)) - offs[rows]
            for t in range(NCH_Q):
                m = tt == t
                ck = si * NCH_Q + t
                cooi_c[ck, pp[m], pos[m]] = cc[m].astype(np.int16)
                coov_c[ck, pp[m], pos[m]] = sub[pp[m], t, cc[m]]
        coov_dev = np.ascontiguousarray(coov_c.transpose(1, 0, 2)).reshape(128, NCK * P_PAD)
        cooi_dev = np.ascontiguousarray(cooi_c.transpose(1, 0, 2)).reshape(128, NCK * P_PAD)

        h0t_c = np.ascontiguousarray(
            u_pad[:, sl].reshape(B, WLOC, 128).transpose(2, 1, 0).reshape(128, WLOC * B)
        ).astype(np.float32)

        # X rows (ctl, u, -, ones) in the partition-spread layout:
        # x4bP[f*32+r, g*1280 + c*16 + t] = X_f[group g, col c*512 + r*16 + t]
        x4bP_c = np.zeros((128, NG * XW_HOST), np.float16)
        for f, row in ((0, ctl_pad[:, sl]), (1, u_pad[:, sl]), (3, None)):
            if row is None:
                flat = np.ones(NLOC * B, np.float32)
            else:
                flat = np.ascontiguousarray(row.T).reshape(-1)  # [n*B + b]
            v = flat.reshape(NG, 80, 32, 16).transpose(2, 0, 1, 3).reshape(32, NG * XW_HOST)
            x4bP_c[f * 32 : (f + 1) * 32] = v.astype(np.float16)

        in_maps.append(
            {
                "a2T": a2T_c,
                "h0sig": h0sig_np,
                "h0t": h0t_c,
                "coov": coov_dev,
                "cooi": cooi_dev,
                "x4bP": x4bP_c,
                "w1x": w1x_np,
                "w2c": w2c_np,
                "b2v": b2v_np,
                "alph": alph_np,
            }
        )
    return in_maps


XW_HOST = XCOLS // 32


def kernel(
    ctl_base,
    u_raw,
    g_logits,
    alpha_logits,
    cell_emb,
    W1,
    b1,
    W2,
    b2,
    edge_val,
    edge_src,
    edge_dst,
    cell_idx,
):
    from concourse.bass_utils import run_bass_kernel_spmd

    args = dict(
        ctl_base=np.asarray(ctl_base), u_raw=np.asarray(u_raw),
        g_logits=np.asarray(g_logits), alpha_logits=np.asarray(alpha_logits),
        cell_emb=np.asarray(cell_emb), W1=np.asarray(W1), b1=np.asarray(b1),
        W2=np.asarray(W2), b2=np.asarray(b2), edge_val=np.asarray(edge_val),
        edge_src=np.asarray(edge_src), edge_dst=np.asarray(edge_dst),
        cell_idx=np.asarray(cell_idx),
    )
    in_maps = _host_prep(**args)
    nc = _build_program()
    _CACHE["in_maps"] = in_maps
    res = run_bass_kernel_spmd(nc, in_maps, core_ids=list(range(CORES)))
    out = np.concatenate([res.results[c]["y"] for c in range(CORES)], axis=1)
    return np.ascontiguousarray(out[:, :N]).astype(np.float32)


# revision 52
# speedup vs baseline: 12126.6391x; 1.0021x over previous
"""GNN message passing (nn_OPID_78769700208710) on 8 TRN2 NeuronCores.

The 6-relation edge lists are combined on host into one sparse operator
(w[e] = sign_r * softplus(g_r) * val[e]) and materialized dense in fp16.
Propagation h_{k+1} = a_k*h0 + (1-a_k)*(h @ A) runs 6 steps on device;
decode (per-node MLP) is fused into step 6.

Sharding: destination-column model parallelism — core c owns dst columns
[c*2560, (c+1)*2560).  A's slice is stored dst-major as 20 "column panels"
[128 src x (160 win * 128 dst)] so each dst block's PSUM accumulator
completes early in the step, letting the inter-step h exchange and the
decode overlap compute.

Panel materialization is split between two engines running concurrently:
  - DMA streams most quarter-panels from HBM, and
  - GPSIMD local_scatter densifies the rest from an SBUF-resident COO copy
    of the same fp16 values (A is ~0.6% dense so the COO fits in SBUF),
    adding a second materialization pipe on an otherwise idle engine.

h exchange: each step's new h slice is AllGathered in 4 groups of 5 dst
blocks.  Window order is permuted (sigma) to group-arrival order, and deep
quarter/PSUM buffering absorbs the exchange latency at step starts.

Decode: the step-6 alpha-mix is folded into W1 on host (h0 = u_raw), so
msg6 feeds the MLP directly.  X uses a partition-spread layout
([128, 1280] per step: partition f*32+r holds X_f for columns r*16+t of
each 512-column chunk) with a zero-padded ct=128 stationary W1, so X fills
are cheap 128-partition DMAs.  cell_emb passes linearly through W2 and is
folded (with b2) into a per-partition bias.
"""

import numpy as np

N = 20000
NP = 20480          # padded nodes: 160 windows * 128
W = 160             # src windows of 128
B = 64              # batch
CORES = 8
NLOC = NP // CORES  # 2560 dst nodes per core
WLOC = NLOC // 128  # 20 dst blocks per core
H = 64
STEPS = 6
SIGNS = (1.0, -1.0, 1.0, -1.0, 1.0, -1.0)

NG = 4              # decode groups per step (xP/y granularity)
BPG = WLOC // NG    # 5 dst blocks per decode group
NG_EX = 2           # exchange groups per step (collective granularity)
BPG_EX = WLOC // NG_EX  # 10 dst blocks per exchange group
# position i in sigma-order <-> global window SIGMA[i]
SIGMA = [
    20 * p + BPG_EX * g + j
    for g in range(NG_EX)
    for p in range(CORES)
    for j in range(BPG_EX)
]

NQ = 4              # quarters per column panel
QW = W // NQ        # 40 window-positions per quarter
CHUNK = 1280        # local_scatter chunk (columns) — must be <= 2046
NCH_Q = QW * 128 // CHUNK  # 4 chunks per quarter
P_PAD = 32          # padded nonzeros per (chunk, partition-row)

# quarters (k = d*NQ + q) with COO staged in SBUF for gpsimd local_scatter.
# Steps 1-5: only blocks d%10 in {3..9} — the Pool engine is blocked ~48us by
# the AllGather right when blocks d%10 in {0,1,2} of the next group
# materialize, so those are always DMA-sourced.  Step 6 has no collectives:
# spread set, sized so DMA and gpsimd finish together.
SCAT_6 = [k for k in range(WLOC * NQ) if (k * 42) % (WLOC * NQ) < 42]
_OK6 = [k for k in SCAT_6 if (k // NQ) % BPG_EX >= 2]
SCAT_P = [k for i, k in enumerate(_OK6) if (i * 28) % len(_OK6) < 28]
SCAT_KS = sorted(set(SCAT_P) | set(SCAT_6))
SCAT_RANK = {kq: i for i, kq in enumerate(SCAT_KS)}
SET_P = frozenset(SCAT_P)
SET_6 = frozenset(SCAT_6)
NCK = len(SCAT_KS) * NCH_Q  # COO chunks per core

XCOLS = BPG * 128 * B       # 40960 X columns per group
YC_G = XCOLS // 128         # 320 y (ps2) columns per group

_CACHE = {}

import os as _os
_NO_SCAT = _os.environ.get("NO_SCAT", "0") == "1"


def _np_softplus(x):
    return np.log1p(np.exp(-np.abs(x))) + np.maximum(x, 0.0)


def _np_sigmoid(x):
    return 1.0 / (1.0 + np.exp(-x))


def _build_program(compile_=True, debug=False):
    key = ("nc2", debug)
    if key in _CACHE:
        return _CACHE[key]

    import concourse.bacc as bacc
    import concourse.mybir as mybir
    from concourse import tile

    f16 = mybir.dt.float16
    f32 = mybir.dt.float32
    i16 = mybir.dt.int16
    AF = mybir.ActivationFunctionType
    OP = mybir.AluOpType

    nc = bacc.Bacc(
        "TRN2",
        target_bir_lowering=False,
        debug=False,
        enable_asserts=False,
        num_devices=CORES,
    )

    a2T = nc.dram_tensor("a2T", [WLOC, 128, W * 128], f16, kind="ExternalInput")
    h0sig = nc.dram_tensor("h0sig", [128, W * B], f16, kind="ExternalInput")
    h0t = nc.dram_tensor("h0t", [128, WLOC * B], f32, kind="ExternalInput")
    coov = nc.dram_tensor("coov", [128, NCK * P_PAD], f16, kind="ExternalInput")
    cooi = nc.dram_tensor("cooi", [128, NCK * P_PAD], i16, kind="ExternalInput")
    x4bP = nc.dram_tensor("x4bP", [128, NG * XCOLS // 32], f16, kind="ExternalInput")
    w1x = nc.dram_tensor("w1x", [128, 32 * H], f16, kind="ExternalInput")
    w2c = nc.dram_tensor("w2c", [H, 1], f16, kind="ExternalInput")
    b2v = nc.dram_tensor("b2v", [128, NG * YC_G], f32, kind="ExternalInput")
    alph = nc.dram_tensor("alph", [128, 2 * STEPS], f32, kind="ExternalInput")
    y = nc.dram_tensor("y", [B, NLOC], f32, kind="ExternalOutput")
    if debug:
        dbg_h = nc.dram_tensor("dbg_h", [STEPS - 1, 128, W * B], f16, kind="ExternalOutput")
        dbg_x = nc.dram_tensor("dbg_x", [128, NG * XCOLS // 32], f16, kind="ExternalOutput")

    XW = XCOLS // 32            # 1280 xP columns per group

    with tile.TileContext(nc) as tc:
        with (
            tc.tile_pool(name="const", bufs=1) as constp,
            tc.tile_pool(name="qp", bufs=11) as qpool,
            tc.tile_pool(name="tmp", bufs=2) as tmpp,
            tc.tile_pool(name="big", bufs=2) as bip,
            tc.tile_pool(name="hds", bufs=2) as hdsp,
            tc.tile_pool(name="ysb", bufs=2) as ysbp,
            tc.tile_pool(name="accps", bufs=5, space="PSUM") as accp,
            tc.tile_pool(name="psa", bufs=2, space="PSUM") as psap,
            tc.tile_pool(name="ps2", bufs=1, space="PSUM") as ps2p,
            tc.tile_pool(name="dram", bufs=2, space="DRAM") as dramp,
        ):
            # ---- persistent SBUF state ----
            # double-buffered by step parity: step k reads set k%2, the
            # in-step exchange writes set (k+1)%2 for the next step
            h_slab = [
                [
                    constp.tile([128, QW * B], f16, tag=f"hsl{s}{g}", name=f"hsl{s}{g}")
                    for g in range(NG)
                ]
                for s in range(2)
            ]
            h0t_sb = constp.tile([128, WLOC * B], f32, tag="h0t")
            coov_sb = constp.tile([128, NCK * P_PAD], f16, tag="coov")
            cooi_sb = constp.tile([128, NCK * P_PAD], i16, tag="cooi")
            alph_sb = constp.tile([128, 2 * STEPS], f32, tag="alph")
            w1x_sb = constp.tile([128, 32 * H], f16, tag="w1x")
            w2c_sb = constp.tile([H, 1], f16, tag="w2c")
            b2v_sb = constp.tile([128, NG * YC_G], f32, tag="b2v")
            xP = constp.tile([128, NG * XW], f16, tag="xP")

            for g in range(NG):
                nc.sync.dma_start(
                    h_slab[0][g][:], h0sig.ap()[:, g * QW * B : (g + 1) * QW * B]
                )
            nc.sync.dma_start(h0t_sb[:], h0t.ap())
            nc.sync.dma_start(coov_sb[:], coov.ap())
            nc.sync.dma_start(cooi_sb[:], cooi.ap())
            nc.sync.dma_start(alph_sb[:], alph.ap())
            nc.sync.dma_start(w1x_sb[:], w1x.ap())
            nc.sync.dma_start(w2c_sb[:], w2c.ap())
            nc.sync.dma_start(b2v_sb[:], b2v.ap())
            nc.sync.dma_start(xP[:], x4bP.ap())

            # DRAM bounce for the step-6 msg -> X row reshuffle
            xrow2 = dramp.tile([WLOC, 128, B], f16, tag="xrow2", bufs=1)

            for k in range(STEPS):
                for d in range(WLOC):
                    g = d // BPG
                    acc = accp.tile([128, B], f32, tag="acc")
                    for q in range(NQ):
                        kq = d * NQ + q
                        qt = qpool.tile([128, QW * 128], f16, tag="qp")
                        use_scat = (not _NO_SCAT) and kq in (
                            SET_6 if k == STEPS - 1 else SET_P
                        )
                        if use_scat:
                            ck0 = SCAT_RANK[kq] * NCH_Q
                            for t in range(NCH_Q):
                                ck = ck0 + t
                                nc.gpsimd.local_scatter(
                                    qt[:, t * CHUNK : (t + 1) * CHUNK],
                                    coov_sb[:, ck * P_PAD : (ck + 1) * P_PAD],
                                    cooi_sb[:, ck * P_PAD : (ck + 1) * P_PAD],
                                    128,
                                    CHUNK,
                                    P_PAD,
                                )
                        else:
                            nc.sync.dma_start(
                                qt[:], a2T.ap()[d][:, q * QW * 128 : (q + 1) * QW * 128]
                            )
                        for i in range(QW):
                            nc.tensor.matmul(
                                acc[:],
                                lhsT=qt[:, i * 128 : (i + 1) * 128],
                                rhs=h_slab[k % 2][q][:, i * B : (i + 1) * B],
                                start=(q == 0 and i == 0),
                                stop=(q == NQ - 1 and i == QW - 1),
                            )

                    if k < STEPS - 1:
                        # epilogue: h_new = a*h0 + (1-a)*msg, fp16
                        if d % BPG_EX == 0:
                            bi_sb = bip.tile([128, BPG_EX * B], f16, tag="bi")
                        h0a = tmpp.tile([128, B], f32, tag="h0a")
                        nc.scalar.activation(
                            h0a[:],
                            h0t_sb[:, d * B : (d + 1) * B],
                            AF.Copy,
                            scale=alph_sb[:, k : k + 1],
                        )
                        nc.vector.scalar_tensor_tensor(
                            bi_sb[:, (d % BPG_EX) * B : (d % BPG_EX + 1) * B],
                            acc[:],
                            alph_sb[:, STEPS + k : STEPS + k + 1],
                            h0a[:],
                            OP.mult,
                            OP.add,
                        )
                        if d % BPG_EX == BPG_EX - 1:
                            ge = d // BPG_EX
                            bi_d = dramp.tile([128, BPG_EX * B], f16, tag="bi_d")
                            bo_d = dramp.tile(
                                [CORES, 128, BPG_EX * B], f16, tag="bo_d"
                            )
                            nc.sync.dma_start(bi_d[:], bi_sb[:])
                            nc.gpsimd.collective_compute(
                                "AllGather",
                                OP.bypass,
                                replica_groups=[list(range(CORES))],
                                ins=[bi_d.opt()],
                                outs=[bo_d.opt()],
                            )
                            # group ge covers positions [80*ge, 80*ge+80) =
                            # slabs 2*ge (cores 0-3) and 2*ge+1 (cores 4-7)
                            for half in range(2):
                                slab = h_slab[(k + 1) % 2][2 * ge + half]
                                nc.sync.dma_start(
                                    slab[:].rearrange("p (c f) -> p c f", c=4),
                                    bo_d[:][4 * half : 4 * half + 4].rearrange(
                                        "c p f -> p c f"
                                    ),
                                )
                                if debug:
                                    nc.sync.dma_start(
                                        dbg_h.ap()[k][
                                            :,
                                            (2 * ge + half)
                                            * QW
                                            * B : (2 * ge + half + 1)
                                            * QW
                                            * B,
                                        ],
                                        slab[:],
                                    )
                    else:
                        # step 6: stage raw msg6 (alpha folded into W1) to
                        # DRAM, then decode this block's 16 X-chunks right
                        # away so only the last block's decode is a tail
                        dl = d % BPG
                        if dl == 0:
                            ps2 = ps2p.tile([128, YC_G], f32, tag="ps2")
                        st16 = tmpp.tile([128, B], f16, tag="st16")
                        nc.scalar.activation(st16[:], acc[:], AF.Copy)
                        nc.sync.dma_start(xrow2[:][d], st16[:])
                        # scatter this block's msg6 into xP (f=2 partitions)
                        nc.sync.dma_start(
                            xP[
                                64:96, g * XW + dl * 256 : g * XW + (dl + 1) * 256
                            ].rearrange("r (c t) -> r c t", t=16),
                            xrow2[:]
                            .rearrange("d p f -> (d p f)")[
                                g * XCOLS + dl * 8192 : g * XCOLS + (dl + 1) * 8192
                            ]
                            .rearrange("(c r t) -> r c t", r=32, t=16),
                        )
                        for c8 in range(dl * 16, (dl + 1) * 16):
                            psA = psap.tile([H, 512], f32, tag="psA")
                            rhs = xP[:, g * XW + c8 * 16 : g * XW + (c8 + 1) * 16]
                            for r in range(32):
                                nc.tensor.matmul(
                                    psA[:, r * 16 : (r + 1) * 16],
                                    lhsT=w1x_sb[:, r * H : (r + 1) * H],
                                    rhs=rhs,
                                    start=True,
                                    stop=True,
                                )
                            hds = hdsp.tile([H, 512], f16, tag="hds")
                            if c8 % 2 == 0:
                                nc.scalar.activation(hds[:], psA[:], AF.Relu)
                            else:
                                nc.vector.tensor_scalar_max(hds[:], psA[:], 0.0)
                            for i in range(4):
                                col = c8 * 4 + i
                                nc.tensor.matmul(
                                    ps2[:, col : col + 1],
                                    lhsT=hds[:, i * 128 : (i + 1) * 128],
                                    rhs=w2c_sb[:],
                                    start=True,
                                    stop=True,
                                )

                        if d % BPG == BPG - 1:
                            ysb = ysbp.tile([128, YC_G], f32, tag="ysb")
                            nc.vector.scalar_tensor_tensor(
                                ysb[:],
                                ps2[:],
                                1.0,
                                b2v_sb[:, g * YC_G : (g + 1) * YC_G],
                                OP.mult,
                                OP.add,
                            )
                            if debug:
                                nc.sync.dma_start(
                                    dbg_x.ap()[:, g * XW : (g + 1) * XW],
                                    xP[:, g * XW : (g + 1) * XW],
                                )
                            ysp = y.ap().rearrange("b (f t) -> f t b", t=2)
                            for t in range(2):
                                nc.sync.dma_start(
                                    ysp[g * YC_G : (g + 1) * YC_G][:, t, :].rearrange(
                                        "f b -> b f"
                                    ),
                                    ysb[t * 64 : (t + 1) * 64, :],
                                )

    if compile_:
        nc.compile()
    _CACHE[key] = nc
    return nc


def _host_prep(ctl_base, u_raw, g_logits, alpha_logits, cell_emb,
               W1, b1, W2, b2, edge_val, edge_src, edge_dst, cell_idx):
    g = _np_softplus(np.asarray(g_logits, np.float64))
    alphas = _np_sigmoid(np.asarray(alpha_logits, np.float64))

    A = np.zeros((NP, NP), np.float32)
    for r in range(6):
        w = (SIGNS[r] * g[r]) * np.asarray(edge_val[r], np.float64)
        np.add.at(A, (edge_src[r], edge_dst[r]), w.astype(np.float32))

    u_pad = np.zeros((B, NP), np.float32)
    u_pad[:, :N] = u_raw
    ctl_pad = np.zeros((B, NP), np.float32)
    ctl_pad[:, :N] = ctl_base

    sig = np.asarray(SIGMA)
    # h0 in sigma window layout: [p, i*B+b] = u[b, SIGMA[i]*128+p]
    h0sig_np = np.ascontiguousarray(
        u_pad.reshape(B, W, 128)[:, sig, :].transpose(2, 1, 0).reshape(128, W * B)
    ).astype(np.float16)

    alph_np = np.zeros((128, 2 * STEPS), np.float32)
    alph_np[:, :STEPS] = alphas.astype(np.float32)
    alph_np[:, STEPS:] = (1.0 - alphas).astype(np.float32)

    a5 = alphas[STEPS - 1]
    w1r = np.zeros((4, H), np.float16)
    w1r[0] = W1[0].astype(np.float16)
    w1r[1] = (W1[1].astype(np.float64) + a5 * W1[2].astype(np.float64)).astype(np.float16)
    w1r[2] = ((1.0 - a5) * W1[2].astype(np.float64)).astype(np.float16)
    w1r[3] = b1.astype(np.float16)
    # zero-padded ct=128 stationary operand: w1x[f*32+rho, r*64+h] nonzero
    # only when rho == r
    w1x_np = np.zeros((128, 32 * H), np.float16)
    for r in range(32):
        for f in range(4):
            w1x_np[f * 32 + r, r * H : (r + 1) * H] = w1r[f]

    w2c_np = np.ascontiguousarray(W2.reshape(H, 1)).astype(np.float16)

    # cell_emb passes linearly through W2: per-batch constant + b2, and with
    # n-major X columns the ps2 partition p corresponds to batch b = p % 64.
    ccb = (cell_emb[cell_idx].astype(np.float64) @ W2.astype(np.float64).reshape(H)).astype(np.float32)
    bias_p = (ccb[np.arange(128) % B] + np.float32(np.asarray(b2).reshape(-1)[0]))
    b2v_np = np.broadcast_to(bias_p[:, None], (128, NG * YC_G)).copy().astype(np.float32)

    in_maps = []
    for c in range(CORES):
        sl = slice(c * NLOC, (c + 1) * NLOC)
        Acore = A[:, sl].reshape(W, 128, WLOC, 128)[sig]      # [i, p, d, j]
        a2T_c = np.ascontiguousarray(Acore.transpose(2, 1, 0, 3)).reshape(
            WLOC, 128, W * 128
        ).astype(np.float16)

        # COO for the scatter-capable quarters, from the SAME fp16 values
        coov_c = np.zeros((NCK, 128, P_PAD), np.float16)
        cooi_c = -np.ones((NCK, 128, P_PAD), np.int16)
        for si, kq in enumerate(SCAT_KS):
            d, q = divmod(kq, NQ)
            sub = a2T_c[d][:, q * QW * 128 : (q + 1) * QW * 128].reshape(128, NCH_Q, CHUNK)
            pp, tt, cc = np.nonzero(sub)
            rows = pp * NCH_Q + tt
            cnt = np.bincount(rows, minlength=128 * NCH_Q)
            assert cnt.max() <= P_PAD, f"chunk row overflow: {cnt.max()} > {P_PAD}"
            offs = np.zeros(128 * NCH_Q, np.int64)
            np.cumsum(cnt[:-1], out=offs[1:])
            pos = np.arange(len(rows)) - offs[rows]
            for t in range(NCH_Q):
                m = tt == t
                ck = si * NCH_Q + t
                cooi_c[ck, pp[m], pos[m]] = cc[m].astype(np.int16)
                coov_c[ck, pp[m], pos[m]] = sub[pp[m], t, cc[m]]
        coov_dev = np.ascontiguousarray(coov_c.transpose(1, 0, 2)).reshape(128, NCK * P_PAD)
        cooi_dev = np.ascontiguousarray(cooi_c.transpose(1, 0, 2)).reshape(128, NCK * P_PAD)

        h0t_c = np.ascontiguousarray(
            u_pad[:, sl].reshape(B, WLOC, 128).transpose(2, 1, 0).reshape(128, WLOC * B)
        ).astype(np.float32)

        # X rows (ctl, u, -, ones) in the partition-spread layout:
        # x4bP[f*32+r, g*1280 + c*16 + t] = X_f[group g, col c*512 + r*16 + t]
        x4bP_c = np.zeros((128, NG * XW_HOST), np.float16)
        for f, row in ((0, ctl_pad[:, sl]), (1, u_pad[:, sl]), (3, None)):
            if row is None:
                flat = np.ones(NLOC * B, np.float32)
            else:
                flat = np.ascontiguousarray(row.T).reshape(-1)  # [n*B + b]
            v = flat.reshape(NG, 80, 32, 16).transpose(2, 0, 1, 3).reshape(32, NG * XW_HOST)
            x4bP_c[f * 32 : (f + 1) * 32] = v.astype(np.float16)

        in_maps.append(
            {
                "a2T": a2T_c,
                "h0sig": h0sig_np,
                "h0t": h0t_c,
                "coov": coov_dev,
                "cooi": cooi_dev,
                "x4bP": x4bP_c,
                "w1x": w1x_np,
                "w2c": w2c_np,
                "b2v": b2v_np,
                "alph": alph_np,
            }
        )
    return in_maps


XW_HOST = XCOLS // 32


def kernel(
    ctl_base,
    u_raw,
    g_logits,
    alpha_logits,
    cell_emb,
    W1,
    b1,
    W2,
    b2,
    edge_val,
    edge_src,
    edge_dst,
    cell_idx,
):
    from concourse.bass_utils import run_bass_kernel_spmd

    args = dict(
        ctl_base=np.asarray(ctl_base), u_raw=np.asarray(u_raw),
        g_logits=np.asarray(g_logits), alpha_logits=np.asarray(alpha_logits),
        cell_emb=np.asarray(cell_emb), W1=np.asarray(W1), b1=np.asarray(b1),
        W2=np.asarray(W2), b2=np.asarray(b2), edge_val=np.asarray(edge_val),
        edge_src=np.asarray(edge_src), edge_dst=np.asarray(edge_dst),
        cell_idx=np.asarray(cell_idx),
    )
    in_maps = _host_prep(**args)
    nc = _build_program()
    _CACHE["in_maps"] = in_maps
    res = run_bass_kernel_spmd(nc, in_maps, core_ids=list(range(CORES)))
    out = np.concatenate([res.results[c]["y"] for c in range(CORES)], axis=1)
    return np.ascontiguousarray(out[:, :N]).astype(np.float32)
